# revision 35
# baseline (speedup 1.0000x reference)
"""GQA attention block (B=2, N=2048, D=2048, Hq=32, Hkv=8, d=64) on 8 TRN2 NeuronCores.

Sharding: core c = b*4 + hg  (data-parallel over batch b in {0,1}; tensor-parallel
over 4 head-groups hg, each owning 8 q-heads / 2 kv-heads).  Each core computes a
row-parallel partial of the output projection for its batch; the host sums the 4
partials per batch (fp16 partials).

All matmuls run in bf16 (fp32 matmul costs 4 PE cycles/row vs 1 for bf16);
PSUM accumulation stays fp32 and softmax exp reads fp32 PSUM scores.

Engine-balance notes:
 - ScalarE activation tables: phase 1 uses only {Square, Sqrt, Copy} (one
   sqrt_and_others set), phase 2 only {Exp, Copy} (one exp_and_others set) —
   avoids the ~1.3us per ACT_TABLE_LOAD ping-pong between Ln and Exp sets.
 - The two heads of a pair occupy disjoint 64-partition halves, so their K=64
   score matmuls auto-derive disjoint PE row-group tile_positions and run
   CONCURRENTLY (the pair's scoresT land side by side in one [128,1024] PSUM
   tile, one 1024-wide exp per k-tile).
 - PV runs as one 1024-wide bf16 matmul per k-tile (both heads share the kv
   head, V with an appended ones-column produces y plus the softmax
   denominator); 1/den via DVE reciprocal + GpSimd partition_broadcast (no
   PSUM bank, no broadcast matmul).
 - The out-projection is interleaved per 512-token q-chunk and shares the
   scores' PSUM ring; output DMA'd as fp16 partials.
"""

import numpy as np

D_MODEL = 2048
H_Q, H_KV, D_HEAD = 32, 8, 64
B = 2
N = 2048
ROPE_BASE = 10000.0
EPS = 1e-6
NCORES = 8
P = 128


def _modules():
    import sys

    for p in ("/opt/trn_rl_repo",):
        if p not in sys.path:
            sys.path.insert(0, p)
    import concourse.bass as bass
    import concourse.tile as tile
    from concourse import bacc, mybir
    from concourse.masks import make_identity

    return bass, tile, bacc, mybir, make_identity


def build_nc(n_tok=N, causal=True):
    """Build the single-core SPMD Bass program (identical on all 8 cores)."""
    from contextlib import ExitStack

    bass, tile, bacc, mybir, make_identity = _modules()
    f32 = mybir.dt.float32
    f16 = mybir.dt.float16
    bf16 = mybir.dt.bfloat16
    ts = bass.ts
    AF = mybir.ActivationFunctionType
    OP = mybir.AluOpType

    NT = n_tok // P           # token tiles
    DC = D_MODEL // P         # contraction chunks for qkv proj
    QC = n_tok // 512         # query chunks of 512
    NG = n_tok // 512         # x-load groups (512 tokens each)
    assert QC >= 1 and n_tok % 512 == 0

    nc = bacc.Bacc("TRN2", target_bir_lowering=False, debug=False,
                   num_devices=NCORES)

    xT = nc.dram_tensor("xT", [D_MODEL, n_tok], bf16, kind="ExternalInput").ap()
    wqkv = nc.dram_tensor("wqkv", [D_MODEL, 768], bf16, kind="ExternalInput").ap()
    wo = nc.dram_tensor("wo", [512, D_MODEL], bf16, kind="ExternalInput").ap()
    tabq = nc.dram_tensor("tabq", [P, NT, 4, 32], bf16, kind="ExternalInput").ap()
    tabk = nc.dram_tensor("tabk", [P, NT, 4, 32], bf16, kind="ExternalInput").ap()
    out = nc.dram_tensor("out", [n_tok, D_MODEL], f16, kind="ExternalOutput").ap()

    with ExitStack() as ctx:
        tc = ctx.enter_context(tile.TileContext(nc))

        cpool = ctx.enter_context(tc.tile_pool(name="const", bufs=1))
        # persistent activations (all bf16: they feed matmuls), split into
        # per-512-token block tiles so phase-2 consumers only wait on the
        # producer tiles they actually read (whole-tensor tiles would make
        # the first score matmul wait for the LAST phase-1 token tile)
        NB = n_tok // 512
        qfm = [[cpool.tile([P, 512], bf16, name=f"qfm{c}_{b}")
                for b in range(NB)] for c in range(4)]
        kfm = [cpool.tile([P, 512], bf16, name=f"kfm{b}") for b in range(NB)]
        kswap = [cpool.tile([P, 512], bf16, name=f"kswap{b}")
                 for b in range(NB)]
        yfm = [[cpool.tile([P, 512], bf16, name=f"yfm{c}_{b}")
                for b in range(NB)] for c in range(4)]
        vsb = [cpool.tile([P, 130], bf16, name=f"vsb{t}") for t in range(NT)]
        ident = cpool.tile([P, P], bf16, name="ident")
        make_identity(nc, ident[:])
        eps_t = cpool.tile([P, 1], f32, name="eps_t")
        nc.gpsimd.memset(eps_t[:], EPS)
        for t in range(NT):
            nc.gpsimd.memset(vsb[t][:, 64:65], 1.0)
            nc.gpsimd.memset(vsb[t][:, 129:130], 1.0)

        # ---------------- phase 1: qkv + norm + rope + transpose ----------
        with ExitStack() as p1:
            wpool = p1.enter_context(tc.tile_pool(name="wqkv", bufs=1))
            tpool = p1.enter_context(tc.tile_pool(name="tabs", bufs=1))
            xpool = p1.enter_context(tc.tile_pool(name="xg", bufs=3))
            # bufs=3: three token tiles in flight so each tile's serial
            # rmsnorm/rope/transpose chain overlaps the next tiles' matmuls
            wkk = p1.enter_context(tc.tile_pool(name="qkvwork", bufs=3))
            qkv_ps = p1.enter_context(
                tc.tile_pool(name="qkvpsum", bufs=2, space="PSUM"))
            # 4 bufs: the transpose->copy ring cycles ~0.5us per slot; with
            # only 2 the in-order tensor queue blocks at every 3rd transpose
            tp_ps = p1.enter_context(
                tc.tile_pool(name="tppsum", bufs=4, space="PSUM"))

            # x group 0 + first weight chunk issued first so the first qkv
            # matmul can start as early as possible (wq split into 4 tiles)
            xgs = {}
            xgs[0] = xpool.tile([P, DC, 256], bf16, tag="xg", name="xg0")
            nc.sync.dma_start(
                xgs[0][:], xT[:, ts(0, 256)].rearrange("(o p) t -> p o t", p=P))
            wq_sb = [wpool.tile([P, 4, 768], bf16, name=f"wq{i}")
                     for i in range(4)]
            wq_src = wqkv.rearrange("(o p) r -> p o r", p=P)
            for i in range(4):
                nc.sync.dma_start(wq_sb[i][:], wq_src[:, 4 * i:4 * i + 4, :])
            tq = tpool.tile([P, NT, 4, 32], bf16)
            nc.sync.dma_start(tq[:], tabq)
            tk = tpool.tile([P, NT, 4, 32], bf16)
            nc.sync.dma_start(tk[:], tabk)

            # PE warmup: dummy transposes into the tp ring promote the HAM
            # clock gate to 8/8 (2.4 GHz) and keep the PE busy while the
            # first x/w DMAs land (cold PE runs at 1.2 GHz; the activity
            # window is ~3.4us, so idling through the DMAs would re-demote).
            for wu in range(45):
                wt = tp_ps.tile([P, P], bf16, tag="tp")
                nc.tensor.transpose(wt[:], ident[:], ident[:])

            for g in range(2 * NG):
                if g not in xgs:
                    xgs[g] = xpool.tile([P, DC, 256], bf16, tag="xg",
                                        name=f"xg{g}")
                    nc.sync.dma_start(
                        xgs[g][:],
                        xT[:, ts(g, 256)].rearrange("(o p) t -> p o t", p=P))
                xg = xgs[g]
                for lt in range(2):
                    tt = g * 2 + lt
                    ps = qkv_ps.tile([P, 768], f32, tag="qkv")
                    for dc in range(DC):
                        lhsT = xg[:, dc, ts(lt, P)]
                        wsl = wq_sb[dc // 4][:, dc % 4]
                        nc.tensor.matmul(ps[:, 0:512], lhsT, wsl[:, 0:512],
                                         start=(dc == 0), stop=(dc == DC - 1))
                        nc.tensor.matmul(ps[:, 512:768], lhsT, wsl[:, 512:768],
                                         start=(dc == 0), stop=(dc == DC - 1))
                    # --- rmsnorm (Square+Sqrt stay in the sqrt table set) ---
                    sq = wkk.tile([P, 640], f32, tag="sq")
                    nc.scalar.activation(sq[:], ps[:, 0:640], AF.Square)
                    ssq = wkk.tile([P, 10], f32, tag="ssq")
                    nc.vector.reduce_sum(
                        ssq[:], sq[:].rearrange("p (h d) -> p h d", d=64),
                        axis=mybir.AxisListType.X)
                    sd = wkk.tile([P, 10], f32, tag="sd")
                    nc.scalar.activation(sd[:], ssq[:], AF.Sqrt,
                                         bias=eps_t[:], scale=1.0 / 64)
                    rs = wkk.tile([P, 10], f32, tag="rs")
                    nc.vector.reciprocal(rs[:], sd[:])
                    qn = wkk.tile([P, 512], bf16, tag="qn")
                    nc.vector.tensor_tensor(
                        qn[:].rearrange("p (h d) -> p h d", d=64),
                        ps[:, 0:512].rearrange("p (h d) -> p h d", d=64),
                        rs[:, 0:8, None].to_broadcast([P, 8, 64]), OP.mult)
                    kn = wkk.tile([P, 128], bf16, tag="kn")
                    nc.vector.tensor_tensor(
                        kn[:].rearrange("p (h d) -> p h d", d=64),
                        ps[:, 512:640].rearrange("p (h d) -> p h d", d=64),
                        rs[:, 8:10, None].to_broadcast([P, 2, 64]), OP.mult)
                    # --- v copy (ones cols at 64/129; one strided ACT) ---
                    nc.scalar.activation(
                        vsb[tt][:, 0:130].rearrange(
                            "p (j q) -> p j q", q=65)[:, :, 0:64],
                        ps[:, 640:768].rearrange("p (j q) -> p j q", q=64),
                        AF.Copy)
                    # --- rope: 3 DVE ops per tensor via host-folded tables
                    # tab rows are [A, B, C, -D]; viewed as [P, 2, 2, 32] the
                    # pairs are (A,C) and (B,-D), so
                    # dv = t1*(A,C) - t2*(B,-D) = (t1*A - t2*B | t1*C + t2*D)
                    qr = wkk.tile([P, 512], bf16, tag="qr")
                    kr = wkk.tile([P, 128], bf16, tag="kr")
                    for (src, dst, tab, nh) in ((qn, qr, tq, 8), (kn, kr, tk, 2)):
                        sv = src[:].rearrange("p (h d) -> p h d", d=64)
                        dv = dst[:].rearrange("p (h two f) -> p h two f",
                                              two=2, f=32)
                        tabv = tab[:, tt].rearrange("p (g two) f -> p two g f",
                                                    two=2)
                        t1 = sv[:, :, None, 0:32].to_broadcast([P, nh, 2, 32])
                        t2 = sv[:, :, None, 32:64].to_broadcast([P, nh, 2, 32])
                        AC = tabv[:, 0:1, :, :].to_broadcast([P, nh, 2, 32])
                        BD = tabv[:, 1:2, :, :].to_broadcast([P, nh, 2, 32])
                        u13 = wkk.tile([P, nh, 2, 32], bf16, tag=f"u13_{nh}")
                        u24 = wkk.tile([P, nh, 2, 32], bf16, tag=f"u24_{nh}")
                        nc.vector.tensor_tensor(u13[:], t1, AC, OP.mult)
                        nc.vector.tensor_tensor(u24[:], t2, BD, OP.mult)
                        nc.vector.tensor_tensor(dv, u13[:], u24[:],
                                                OP.subtract)
                    # --- transpose to feature-major (copies on ScalarE) ---
                    tb, tc_ = tt // 4, ts(tt % 4, P)
                    for rc in range(4):
                        pt = tp_ps.tile([P, P], bf16, tag="tp")
                        nc.tensor.transpose(pt[:], qr[:, ts(rc, P)], ident[:])
                        nc.scalar.activation(qfm[rc][tb][:, tc_], pt[:],
                                             AF.Copy)
                    pt = tp_ps.tile([P, P], bf16, tag="tp")
                    nc.tensor.transpose(pt[:], kr[:], ident[:])
                    nc.scalar.activation(kfm[tb][:, tc_], pt[:], AF.Copy)
                    # kswap: partition halves exchanged, built per tile
                    nc.scalar.activation(kswap[tb][64:128, tc_], pt[0:64, :],
                                         AF.Copy)
                    nc.scalar.activation(kswap[tb][0:64, tc_], pt[64:128, :],
                                         AF.Copy)
            # keep the PE clock warm through the last tiles' scalar/vector
            # drain: the tp-ring WAR spaces these pulses at the copy pace
            for wu in range(16):
                wt = tp_ps.tile([P, P], bf16, tag="tp")
                nc.tensor.transpose(wt[:], ident[:], ident[:])

        # ---------------- phase 2: attention + out projection ------------
        wopool = ctx.enter_context(tc.tile_pool(name="wo", bufs=1))
        wo_sb = wopool.tile([P, 4, D_MODEL], bf16, name="wo_sb")
        nc.sync.dma_start(wo_sb[:], wo.rearrange("(o p) d -> p o d", p=P))
        with ExitStack() as p2:
            epool = p2.enter_context(tc.tile_pool(name="exp", bufs=4))
            npool = p2.enter_context(tc.tile_pool(name="nrm", bufs=2))
            opool = p2.enter_context(tc.tile_pool(name="osb", bufs=3))
            s_ps = p2.enter_context(
                tc.tile_pool(name="spsum", bufs=2, space="PSUM"))
            y_ps = p2.enter_context(
                tc.tile_pool(name="ypsum", bufs=2, space="PSUM"))

            # out-projection emission: groups for q-chunk qc are spread
            # through qc+1's attention stream so their matmuls fill the
            # tensor-engine gaps of the exp-bound kt pipeline (in-order
            # engine queues: the filler must sit between the stalls).
            pending = []          # (token tile, output half) groups
            emit_ctr = [0]

            def emit_ogroup():
                t, og = pending.pop(0)
                ps_o = s_ps.tile([P, 1024], f32, tag="s")
                for oc2 in range(2):
                    for yc in range(4):
                        nc.tensor.matmul(
                            ps_o[:, ts(oc2, 512)],
                            yfm[yc][t // 4][:, ts(t % 4, P)],
                            wo_sb[:, yc, 1024 * og + 512 * oc2:
                                  1024 * og + 512 * (oc2 + 1)],
                            start=(yc == 0), stop=(yc == 3))
                ob = opool.tile([P, 1024], f16, tag="ob")
                nc.vector.tensor_copy(ob[:], ps_o[:])
                nc.sync.dma_start(out[ts(t, P), ts(og, 1024)], ob[:])

            # `held` carries the not-yet-emitted PV (+ pair finalizer) of the
            # previous k-tile ACROSS pair boundaries, so scores/exp of the
            # next pair keep both engines fed while the last PV of the
            # previous pair waits on its exp semaphore.
            held = [None]   # (pv_fn, final_fn or None)

            def flush_held():
                if held[0] is None:
                    return
                pv_fn, final_fn = held[0]
                held[0] = None
                pv_fn()
                if final_fn is not None:
                    final_fn()
                emit_ctr[0] += 1
                if pending and emit_ctr[0] % 3 == 2:
                    emit_ogroup()

            for qc in range(QC):
                for c in range(4):
                    kv = c // 2
                    # A = head 2c (partitions 0:64), B = head 2c+1 (64:128)
                    ksA = kfm if kv == 0 else kswap
                    ksB = kswap if kv == 0 else kfm
                    vsl = slice(65 * kv, 65 * kv + 65)
                    nkt = 4 * qc + 4 if causal else 4 * QC
                    ps_y = y_ps.tile([65, 1024], f32, tag="y")

                    def emit_pv(kt, eg, o, ps_y=ps_y, vsl=vsl, nkt=nkt):
                        nc.tensor.matmul(
                            ps_y[:, o:512], vsb[kt][:, vsl], eg[:, o:512],
                            start=(kt == 0), stop=(kt == nkt - 1))
                        nc.tensor.matmul(
                            ps_y[:, 512 + o:1024], vsb[kt][:, vsl],
                            eg[:, 512 + o:1024],
                            start=(kt == 0), stop=(kt == nkt - 1))

                    def normalize(ps_y=ps_y, c=c, qc=qc):
                        # 1/den via DVE recip + GpSimd partition broadcast
                        # (recip can't read PSUM; vector copy bounces row 64)
                        draw = npool.tile([1, 1024], f32, tag="draw")
                        nc.vector.tensor_copy(draw[0:1, :], ps_y[64:65, :])
                        rec = npool.tile([1, 1024], f32, tag="rec")
                        nc.vector.reciprocal_approx_fast(rec[0:1, :],
                                                         draw[0:1, :])
                        rexp = npool.tile([64, 1024], f32, tag="rexp")
                        nc.gpsimd.partition_broadcast(rexp[:], rec[0:1, :],
                                                      channels=64)
                        nc.vector.tensor_tensor(yfm[c][qc][0:64, :],
                                                ps_y[0:64, 0:512],
                                                rexp[:, 0:512], OP.mult)
                        nc.vector.tensor_tensor(yfm[c][qc][64:128, :],
                                                ps_y[0:64, 512:1024],
                                                rexp[:, 512:1024], OP.mult)

                    for kt in range(nkt):
                        jl = kt - 4 * qc  # >=0 inside the diagonal quad
                        diag = causal and jl >= 0
                        o = 128 * jl if diag else 0
                        ps_s = s_ps.tile([P, 1024], f32, tag="s")
                        eg = epool.tile([P, 1024], bf16, tag="eg")
                        kb, kc = kt // 4, ts(kt % 4, P)
                        nc.tensor.matmul(
                            ps_s[:, o:512],
                            ksA[kb][0:64, kc],
                            qfm[c][qc][0:64, o:512],
                            start=True, stop=True)
                        nc.tensor.matmul(
                            ps_s[:, 512 + o:1024],
                            ksB[kb][64:128, kc],
                            qfm[c][qc][64:128, o:512],
                            start=True, stop=True)
                        if not diag:
                            nc.scalar.activation(eg[:], ps_s[:], AF.Exp)
                        else:
                            nc.scalar.activation(
                                eg[:].rearrange("p (j q) -> p j q",
                                                q=512)[:, :, o:512],
                                ps_s[:].rearrange("p (j q) -> p j q",
                                                  q=512)[:, :, o:512],
                                AF.Exp)
                            # causal triangle at the diagonal 128-col block
                            nc.gpsimd.affine_select(
                                eg[:].rearrange("p (j q) -> p j q",
                                                q=512)[:, :, o:o + 128],
                                eg[:].rearrange("p (j q) -> p j q",
                                                q=512)[:, :, o:o + 128],
                                pattern=[[0, 2], [1, 128]],
                                compare_op=OP.is_ge,
                                fill=0.0,
                                base=0,
                                channel_multiplier=-1)
                        flush_held()
                        is_last = kt == nkt - 1
                        held[0] = (
                            lambda kt=kt, eg=eg, o=o, f=emit_pv: f(kt, eg, o),
                            normalize if is_last else None)
                        # ~107ns LDWEIGHTS pulse every few k-tiles keeps the
                        # HAM activity window non-idle (no PSUM needed), so
                        # exp-bound stretches can't demote the PE to 1.2 GHz
                        if kt % 3 == 1:
                            nc.tensor.ldweights(ident[:])
                # queue this q-chunk's out-projection groups (flushed during
                # qc+1; the final chunk's groups are flushed below)
                for tl in range(4):
                    for og in range(2):
                        pending.append((4 * qc + tl, og))
            flush_held()
            while pending:
                emit_ogroup()

    nc.compile()
    return nc


def _rope_tables(pos, norm_w, scale):
    """Build [P, NT, 4, 32] tables A,B,C,D for out1 = t1*A - t2*B,
    out2 = t1*C + t2*D (NeoX rope with folded norm weight + score scale)."""
    n_tok = pos.shape[0]
    f = np.arange(0, D_HEAD, 2, dtype=np.float64) / D_HEAD
    inv_freq = 1.0 / (ROPE_BASE ** f)                       # [32]
    ang = pos.astype(np.float64)[:, None] * inv_freq[None, :]  # [n, 32]
    cos, sin = np.cos(ang), np.sin(ang)
    w1 = norm_w[:32].astype(np.float64)
    w2 = norm_w[32:].astype(np.float64)
    A = cos * w1 * scale
    Bt = sin * w2 * scale
    C = sin * w1 * scale
    D = cos * w2 * scale
    # D negated: the kernel computes t1*(A,C) - t2*(B,-D) in two fused ops
    tab = np.stack([A, Bt, C, -D], axis=1).astype(np.float32)  # [n, 4, 32]
    return np.ascontiguousarray(
        tab.reshape(n_tok // P, P, 4, 32).transpose(1, 0, 2, 3))


def make_in_maps(x, pos, qkv_w, out_w, q_norm_w, k_norm_w, n_tok=N):
    import ml_dtypes
    bf16 = ml_dtypes.bfloat16

    scale = D_HEAD ** -0.5
    tabq = _rope_tables(pos, q_norm_w, scale).astype(bf16)
    tabk = _rope_tables(pos, k_norm_w, 1.0).astype(bf16)
    wq_all = qkv_w[0:H_Q * D_HEAD].reshape(H_Q, D_HEAD, D_MODEL)
    wk_all = qkv_w[H_Q * D_HEAD:(H_Q + H_KV) * D_HEAD].reshape(
        H_KV, D_HEAD, D_MODEL)
    wv_all = qkv_w[(H_Q + H_KV) * D_HEAD:].reshape(H_KV, D_HEAD, D_MODEL)
    wo_all = out_w.reshape(D_MODEL, H_Q, D_HEAD)

    in_maps = []
    for c in range(NCORES):
        b, hg = divmod(c, 4)
        heads = list(range(8 * hg, 8 * hg + 8))
        kvs = [2 * hg, 2 * hg + 1]
        wsel = np.concatenate([
            wq_all[heads].reshape(512, D_MODEL),
            wk_all[kvs].reshape(128, D_MODEL),
            wv_all[kvs].reshape(128, D_MODEL)], axis=0)    # [768, D]
        in_maps.append({
            "xT": np.ascontiguousarray(x[b].T).astype(bf16),
            "wqkv": np.ascontiguousarray(wsel.T).astype(bf16),
            "wo": np.ascontiguousarray(
                wo_all[:, heads].reshape(D_MODEL, 512).T).astype(bf16),
            "tabq": tabq,
            "tabk": tabk,
        })
    return in_maps


def _reference_host(x, mask, pos, qkv_w, out_w, q_norm_w, k_norm_w):
    """Pure-numpy fallback, used only if the mask is not causal."""
    xx = x.astype(np.float64)
    qkv = xx @ qkv_w.T.astype(np.float64)
    Bsz, Nl, _ = x.shape
    qkv = qkv.reshape(Bsz, Nl, H_Q + 2 * H_KV, D_HEAD).transpose(0, 2, 1, 3)
    q, k, v = (qkv[:, :H_Q], qkv[:, H_Q:H_Q + H_KV], qkv[:, H_Q + H_KV:])

    def rms(t, w):
        var = np.mean(t * t, axis=-1, keepdims=True)
        return t / np.sqrt(var + EPS) * w

    def rope(t):
        f = np.arange(0, D_HEAD, 2) / D_HEAD
        inv = 1.0 / (ROPE_BASE ** f)
        ang = pos.astype(np.float64)[:, None] * inv[None, :]
        cs, sn = np.cos(ang), np.sin(ang)
        t1, t2 = t[..., :32], t[..., 32:]
        return np.concatenate([t1 * cs - t2 * sn, t1 * sn + t2 * cs], axis=-1)

    q, k = rope(rms(q, q_norm_w)), rope(rms(k, k_norm_w))
    qg = q.reshape(Bsz, H_KV, 4, Nl, D_HEAD)
    sc = np.einsum("bhgnd,bhmd->bhgnm", qg, k) * (D_HEAD ** -0.5)
    sc = np.where(mask[None, None, None], -np.inf, sc)
    sc -= sc.max(axis=-1, keepdims=True)
    p = np.exp(sc)
    p /= p.sum(axis=-1, keepdims=True)
    y = np.einsum("bhgnm,bhmd->bhgnd", p, v)
    y = y.reshape(Bsz, H_Q, Nl, D_HEAD).transpose(0, 2, 1, 3).reshape(
        Bsz, Nl, D_MODEL)
    return (y @ out_w.T.astype(np.float64)).astype(np.float32)


_NC_CACHE = {}


def run_on_device(in_maps, n_tok=N, trace=False, trace_kwargs=None):
    import sys
    for p in ("/opt/trn_rl_repo",):
        if p not in sys.path:
            sys.path.insert(0, p)
    from concourse.bass_utils import run_bass_kernel_spmd

    key = n_tok
    if key not in _NC_CACHE:
        _NC_CACHE[key] = build_nc(n_tok)
    nc = _NC_CACHE[key]
    return run_bass_kernel_spmd(
        nc, in_maps, list(range(len(in_maps))), trace=trace,
        **(trace_kwargs or {}))


def kernel(x, mask, pos, qkv_w, out_w, q_norm_w, k_norm_w):
    x = np.asarray(x, dtype=np.float32)
    mask = np.asarray(mask)
    pos = np.asarray(pos)
    causal = bool(
        np.array_equal(mask,
                       np.triu(np.ones((N, N), dtype=bool), k=1)))
    if not causal:
        return _reference_host(x, mask, pos, np.asarray(qkv_w),
                               np.asarray(out_w), np.asarray(q_norm_w),
                               np.asarray(k_norm_w))
    in_maps = make_in_maps(x, pos, np.asarray(qkv_w, dtype=np.float32),
                           np.asarray(out_w, dtype=np.float32),
                           np.asarray(q_norm_w, dtype=np.float32),
                           np.asarray(k_norm_w, dtype=np.float32))
    res = run_on_device(in_maps)
    outs = [r["out"].astype(np.float32) for r in res.results]
    full = np.empty((B, N, D_MODEL), dtype=np.float32)
    for b in range(B):
        full[b] = outs[4 * b] + outs[4 * b + 1] + outs[4 * b + 2] + outs[4 * b + 3]
    return full


# revision 37
# speedup vs baseline: 1.0014x; 1.0014x over previous
"""GQA attention block (B=2, N=2048, D=2048, Hq=32, Hkv=8, d=64) on 8 TRN2 NeuronCores.

Sharding: core c = b*4 + hg  (data-parallel over batch b in {0,1}; tensor-parallel
over 4 head-groups hg, each owning 8 q-heads / 2 kv-heads).  Each core computes a
row-parallel partial of the output projection for its batch; the host sums the 4
partials per batch (fp16 partials).

All matmuls run in bf16 (fp32 matmul costs 4 PE cycles/row vs 1 for bf16);
PSUM accumulation stays fp32 and softmax exp reads fp32 PSUM scores.

Engine-balance notes:
 - ScalarE activation tables: phase 1 uses only {Square, Sqrt, Copy} (one
   sqrt_and_others set), phase 2 only {Exp, Copy} (one exp_and_others set) —
   avoids the ~1.3us per ACT_TABLE_LOAD ping-pong between Ln and Exp sets.
 - The two heads of a pair occupy disjoint 64-partition halves, so their K=64
   score matmuls auto-derive disjoint PE row-group tile_positions and run
   CONCURRENTLY (the pair's scoresT land side by side in one [128,1024] PSUM
   tile, one 1024-wide exp per k-tile).
 - PV runs as one 1024-wide bf16 matmul per k-tile (both heads share the kv
   head, V with an appended ones-column produces y plus the softmax
   denominator); 1/den via DVE reciprocal + GpSimd partition_broadcast (no
   PSUM bank, no broadcast matmul).
 - The out-projection is interleaved per 512-token q-chunk and shares the
   scores' PSUM ring; output DMA'd as fp16 partials.
"""

import numpy as np

D_MODEL = 2048
H_Q, H_KV, D_HEAD = 32, 8, 64
B = 2
N = 2048
ROPE_BASE = 10000.0
EPS = 1e-6
NCORES = 8
P = 128


def _modules():
    import sys

    for p in ("/opt/trn_rl_repo",):
        if p not in sys.path:
            sys.path.insert(0, p)
    import concourse.bass as bass
    import concourse.tile as tile
    from concourse import bacc, mybir
    from concourse.masks import make_identity

    return bass, tile, bacc, mybir, make_identity


def build_nc(n_tok=N, causal=True):
    """Build the single-core SPMD Bass program (identical on all 8 cores)."""
    from contextlib import ExitStack

    bass, tile, bacc, mybir, make_identity = _modules()
    f32 = mybir.dt.float32
    f16 = mybir.dt.float16
    bf16 = mybir.dt.bfloat16
    ts = bass.ts
    AF = mybir.ActivationFunctionType
    OP = mybir.AluOpType

    NT = n_tok // P           # token tiles
    DC = D_MODEL // P         # contraction chunks for qkv proj
    QC = n_tok // 512         # query chunks of 512
    NG = n_tok // 512         # x-load groups (512 tokens each)
    assert QC >= 1 and n_tok % 512 == 0

    nc = bacc.Bacc("TRN2", target_bir_lowering=False, debug=False,
                   num_devices=NCORES)

    xT = nc.dram_tensor("xT", [D_MODEL, n_tok], bf16, kind="ExternalInput").ap()
    wqkv = nc.dram_tensor("wqkv", [D_MODEL, 768], bf16, kind="ExternalInput").ap()
    wo = nc.dram_tensor("wo", [512, D_MODEL], bf16, kind="ExternalInput").ap()
    tabq = nc.dram_tensor("tabq", [P, NT, 4, 32], bf16, kind="ExternalInput").ap()
    tabk = nc.dram_tensor("tabk", [P, NT, 4, 32], bf16, kind="ExternalInput").ap()
    out = nc.dram_tensor("out", [n_tok, D_MODEL], f16, kind="ExternalOutput").ap()

    with ExitStack() as ctx:
        tc = ctx.enter_context(tile.TileContext(nc))

        cpool = ctx.enter_context(tc.tile_pool(name="const", bufs=1))
        # persistent activations (all bf16: they feed matmuls), split into
        # per-512-token block tiles so phase-2 consumers only wait on the
        # producer tiles they actually read (whole-tensor tiles would make
        # the first score matmul wait for the LAST phase-1 token tile)
        NB = n_tok // 512
        qfm = [[cpool.tile([P, 512], bf16, name=f"qfm{c}_{b}")
                for b in range(NB)] for c in range(4)]
        kfm = [cpool.tile([P, 512], bf16, name=f"kfm{b}") for b in range(NB)]
        kswap = [cpool.tile([P, 512], bf16, name=f"kswap{b}")
                 for b in range(NB)]
        yfm = [[cpool.tile([P, 512], bf16, name=f"yfm{c}_{b}")
                for b in range(NB)] for c in range(4)]
        vsb = [cpool.tile([P, 130], bf16, name=f"vsb{t}") for t in range(NT)]
        ident = cpool.tile([P, P], bf16, name="ident")
        make_identity(nc, ident[:])
        eps_t = cpool.tile([P, 1], f32, name="eps_t")
        nc.gpsimd.memset(eps_t[:], EPS)
        for t in range(NT):
            nc.gpsimd.memset(vsb[t][:, 64:65], 1.0)
            nc.gpsimd.memset(vsb[t][:, 129:130], 1.0)

        # ---------------- phase 1: qkv + norm + rope + transpose ----------
        with ExitStack() as p1:
            wpool = p1.enter_context(tc.tile_pool(name="wqkv", bufs=1))
            tpool = p1.enter_context(tc.tile_pool(name="tabs", bufs=1))
            xpool = p1.enter_context(tc.tile_pool(name="xg", bufs=3))
            # bufs=3: three token tiles in flight so each tile's serial
            # rmsnorm/rope/transpose chain overlaps the next tiles' matmuls
            wkk = p1.enter_context(tc.tile_pool(name="qkvwork", bufs=3))
            qkv_ps = p1.enter_context(
                tc.tile_pool(name="qkvpsum", bufs=2, space="PSUM"))
            # 4 bufs: the transpose->copy ring cycles ~0.5us per slot; with
            # only 2 the in-order tensor queue blocks at every 3rd transpose
            tp_ps = p1.enter_context(
                tc.tile_pool(name="tppsum", bufs=4, space="PSUM"))

            # x group 0 + first weight chunk issued first so the first qkv
            # matmul can start as early as possible (wq split into 4 tiles)
            xgs = {}
            xgs[0] = xpool.tile([P, DC, 256], bf16, tag="xg", name="xg0")
            nc.sync.dma_start(
                xgs[0][:], xT[:, ts(0, 256)].rearrange("(o p) t -> p o t", p=P))
            wq_sb = [wpool.tile([P, 4, 768], bf16, name=f"wq{i}")
                     for i in range(4)]
            wq_src = wqkv.rearrange("(o p) r -> p o r", p=P)
            for i in range(4):
                nc.sync.dma_start(wq_sb[i][:], wq_src[:, 4 * i:4 * i + 4, :])
            tq = tpool.tile([P, NT, 4, 32], bf16)
            nc.sync.dma_start(tq[:], tabq)
            tk = tpool.tile([P, NT, 4, 32], bf16)
            nc.sync.dma_start(tk[:], tabk)

            # PE warmup: dummy transposes into the tp ring promote the HAM
            # clock gate to 8/8 (2.4 GHz) and keep the PE busy while the
            # first x/w DMAs land (cold PE runs at 1.2 GHz; the activity
            # window is ~3.4us, so idling through the DMAs would re-demote).
            for wu in range(45):
                wt = tp_ps.tile([P, P], bf16, tag="tp")
                nc.tensor.transpose(wt[:], ident[:], ident[:])

            for g in range(2 * NG):
                if g not in xgs:
                    xgs[g] = xpool.tile([P, DC, 256], bf16, tag="xg",
                                        name=f"xg{g}")
                    nc.sync.dma_start(
                        xgs[g][:],
                        xT[:, ts(g, 256)].rearrange("(o p) t -> p o t", p=P))
                xg = xgs[g]
                for lt in range(2):
                    tt = g * 2 + lt
                    ps = qkv_ps.tile([P, 768], f32, tag="qkv")
                    for dc in range(DC):
                        lhsT = xg[:, dc, ts(lt, P)]
                        wsl = wq_sb[dc // 4][:, dc % 4]
                        nc.tensor.matmul(ps[:, 0:512], lhsT, wsl[:, 0:512],
                                         start=(dc == 0), stop=(dc == DC - 1))
                        nc.tensor.matmul(ps[:, 512:768], lhsT, wsl[:, 512:768],
                                         start=(dc == 0), stop=(dc == DC - 1))
                    # --- rmsnorm (Square+Sqrt stay in the sqrt table set) ---
                    sq = wkk.tile([P, 640], f32, tag="sq")
                    nc.scalar.activation(sq[:], ps[:, 0:640], AF.Square)
                    ssq = wkk.tile([P, 10], f32, tag="ssq")
                    nc.vector.reduce_sum(
                        ssq[:], sq[:].rearrange("p (h d) -> p h d", d=64),
                        axis=mybir.AxisListType.X)
                    sd = wkk.tile([P, 10], f32, tag="sd")
                    nc.scalar.activation(sd[:], ssq[:], AF.Sqrt,
                                         bias=eps_t[:], scale=1.0 / 64)
                    rs = wkk.tile([P, 10], f32, tag="rs")
                    nc.vector.reciprocal(rs[:], sd[:])
                    qn = wkk.tile([P, 512], bf16, tag="qn")
                    nc.vector.tensor_tensor(
                        qn[:].rearrange("p (h d) -> p h d", d=64),
                        ps[:, 0:512].rearrange("p (h d) -> p h d", d=64),
                        rs[:, 0:8, None].to_broadcast([P, 8, 64]), OP.mult)
                    kn = wkk.tile([P, 128], bf16, tag="kn")
                    nc.vector.tensor_tensor(
                        kn[:].rearrange("p (h d) -> p h d", d=64),
                        ps[:, 512:640].rearrange("p (h d) -> p h d", d=64),
                        rs[:, 8:10, None].to_broadcast([P, 2, 64]), OP.mult)
                    # --- v copy (ones cols at 64/129; one strided ACT) ---
                    nc.scalar.activation(
                        vsb[tt][:, 0:130].rearrange(
                            "p (j q) -> p j q", q=65)[:, :, 0:64],
                        ps[:, 640:768].rearrange("p (j q) -> p j q", q=64),
                        AF.Copy)
                    # --- rope: 3 DVE ops per tensor via host-folded tables
                    # tab rows are [A, B, C, -D]; viewed as [P, 2, 2, 32] the
                    # pairs are (A,C) and (B,-D), so
                    # dv = t1*(A,C) - t2*(B,-D) = (t1*A - t2*B | t1*C + t2*D)
                    qr = wkk.tile([P, 512], bf16, tag="qr")
                    kr = wkk.tile([P, 128], bf16, tag="kr")
                    for (src, dst, tab, nh) in ((qn, qr, tq, 8), (kn, kr, tk, 2)):
                        sv = src[:].rearrange("p (h d) -> p h d", d=64)
                        dv = dst[:].rearrange("p (h two f) -> p h two f",
                                              two=2, f=32)
                        tabv = tab[:, tt].rearrange("p (g two) f -> p two g f",
                                                    two=2)
                        t1 = sv[:, :, None, 0:32].to_broadcast([P, nh, 2, 32])
                        t2 = sv[:, :, None, 32:64].to_broadcast([P, nh, 2, 32])
                        AC = tabv[:, 0:1, :, :].to_broadcast([P, nh, 2, 32])
                        BD = tabv[:, 1:2, :, :].to_broadcast([P, nh, 2, 32])
                        u13 = wkk.tile([P, nh, 2, 32], bf16, tag=f"u13_{nh}")
                        u24 = wkk.tile([P, nh, 2, 32], bf16, tag=f"u24_{nh}")
                        nc.vector.tensor_tensor(u13[:], t1, AC, OP.mult)
                        nc.vector.tensor_tensor(u24[:], t2, BD, OP.mult)
                        nc.vector.tensor_tensor(dv, u13[:], u24[:],
                                                OP.subtract)
                    # --- transpose to feature-major (copies on ScalarE) ---
                    tb, tc_ = tt // 4, ts(tt % 4, P)
                    for rc in range(4):
                        pt = tp_ps.tile([P, P], bf16, tag="tp")
                        nc.tensor.transpose(pt[:], qr[:, ts(rc, P)], ident[:])
                        nc.scalar.activation(qfm[rc][tb][:, tc_], pt[:],
                                             AF.Copy)
                    pt = tp_ps.tile([P, P], bf16, tag="tp")
                    nc.tensor.transpose(pt[:], kr[:], ident[:])
                    nc.scalar.activation(kfm[tb][:, tc_], pt[:], AF.Copy)
                    # kswap: partition halves exchanged, built per tile
                    nc.scalar.activation(kswap[tb][64:128, tc_], pt[0:64, :],
                                         AF.Copy)
                    nc.scalar.activation(kswap[tb][0:64, tc_], pt[64:128, :],
                                         AF.Copy)
            # phase-transition drain: first a contiguous ~3.8us LDWEIGHTS
            # burst (runs right after the last qkv matmul; sustained activity
            # is what the HAM promotion window needs), then tp-ring-spaced
            # transpose pulses to carry activity through the copy drain
            for wu in range(36):
                nc.tensor.ldweights(ident[:])
            for wu in range(16):
                wt = tp_ps.tile([P, P], bf16, tag="tp")
                nc.tensor.transpose(wt[:], ident[:], ident[:])

        # ---------------- phase 2: attention + out projection ------------
        wopool = ctx.enter_context(tc.tile_pool(name="wo", bufs=1))
        wo_sb = wopool.tile([P, 4, D_MODEL], bf16, name="wo_sb")
        nc.sync.dma_start(wo_sb[:], wo.rearrange("(o p) d -> p o d", p=P))
        with ExitStack() as p2:
            epool = p2.enter_context(tc.tile_pool(name="exp", bufs=4))
            npool = p2.enter_context(tc.tile_pool(name="nrm", bufs=2))
            opool = p2.enter_context(tc.tile_pool(name="osb", bufs=3))
            s_ps = p2.enter_context(
                tc.tile_pool(name="spsum", bufs=2, space="PSUM"))
            y_ps = p2.enter_context(
                tc.tile_pool(name="ypsum", bufs=2, space="PSUM"))

            # out-projection emission: groups for q-chunk qc are spread
            # through qc+1's attention stream so their matmuls fill the
            # tensor-engine gaps of the exp-bound kt pipeline (in-order
            # engine queues: the filler must sit between the stalls).
            pending = []          # (token tile, output half) groups
            emit_ctr = [0]

            def emit_ogroup():
                t, og = pending.pop(0)
                ps_o = s_ps.tile([P, 1024], f32, tag="s")
                for oc2 in range(2):
                    for yc in range(4):
                        nc.tensor.matmul(
                            ps_o[:, ts(oc2, 512)],
                            yfm[yc][t // 4][:, ts(t % 4, P)],
                            wo_sb[:, yc, 1024 * og + 512 * oc2:
                                  1024 * og + 512 * (oc2 + 1)],
                            start=(yc == 0), stop=(yc == 3))
                ob = opool.tile([P, 1024], f16, tag="ob")
                nc.vector.tensor_copy(ob[:], ps_o[:])
                nc.sync.dma_start(out[ts(t, P), ts(og, 1024)], ob[:])

            # `held` carries the not-yet-emitted PV (+ pair finalizer) of the
            # previous k-tile ACROSS pair boundaries, so scores/exp of the
            # next pair keep both engines fed while the last PV of the
            # previous pair waits on its exp semaphore.
            held = [None]   # (pv_fn, final_fn or None)

            def flush_held():
                if held[0] is None:
                    return
                pv_fn, final_fn = held[0]
                held[0] = None
                pv_fn()
                if final_fn is not None:
                    final_fn()
                emit_ctr[0] += 1
                if pending and emit_ctr[0] % 3 == 2:
                    emit_ogroup()

            for qc in range(QC):
                for c in range(4):
                    kv = c // 2
                    # A = head 2c (partitions 0:64), B = head 2c+1 (64:128)
                    ksA = kfm if kv == 0 else kswap
                    ksB = kswap if kv == 0 else kfm
                    vsl = slice(65 * kv, 65 * kv + 65)
                    nkt = 4 * qc + 4 if causal else 4 * QC
                    ps_y = y_ps.tile([65, 1024], f32, tag="y")

                    def emit_pv(kt, eg, o, ps_y=ps_y, vsl=vsl, nkt=nkt):
                        nc.tensor.matmul(
                            ps_y[:, o:512], vsb[kt][:, vsl], eg[:, o:512],
                            start=(kt == 0), stop=(kt == nkt - 1))
                        nc.tensor.matmul(
                            ps_y[:, 512 + o:1024], vsb[kt][:, vsl],
                            eg[:, 512 + o:1024],
                            start=(kt == 0), stop=(kt == nkt - 1))

                    def normalize(ps_y=ps_y, c=c, qc=qc):
                        # 1/den via DVE recip + GpSimd partition broadcast
                        # (recip can't read PSUM; vector copy bounces row 64)
                        draw = npool.tile([1, 1024], f32, tag="draw")
                        nc.vector.tensor_copy(draw[0:1, :], ps_y[64:65, :])
                        rec = npool.tile([1, 1024], f32, tag="rec")
                        nc.vector.reciprocal_approx_fast(rec[0:1, :],
                                                         draw[0:1, :])
                        rexp = npool.tile([64, 1024], f32, tag="rexp")
                        nc.gpsimd.partition_broadcast(rexp[:], rec[0:1, :],
                                                      channels=64)
                        nc.vector.tensor_tensor(yfm[c][qc][0:64, :],
                                                ps_y[0:64, 0:512],
                                                rexp[:, 0:512], OP.mult)
                        nc.vector.tensor_tensor(yfm[c][qc][64:128, :],
                                                ps_y[0:64, 512:1024],
                                                rexp[:, 512:1024], OP.mult)

                    for kt in range(nkt):
                        jl = kt - 4 * qc  # >=0 inside the diagonal quad
                        diag = causal and jl >= 0
                        o = 128 * jl if diag else 0
                        ps_s = s_ps.tile([P, 1024], f32, tag="s")
                        eg = epool.tile([P, 1024], bf16, tag="eg")
                        kb, kc = kt // 4, ts(kt % 4, P)
                        nc.tensor.matmul(
                            ps_s[:, o:512],
                            ksA[kb][0:64, kc],
                            qfm[c][qc][0:64, o:512],
                            start=True, stop=True)
                        nc.tensor.matmul(
                            ps_s[:, 512 + o:1024],
                            ksB[kb][64:128, kc],
                            qfm[c][qc][64:128, o:512],
                            start=True, stop=True)
                        if not diag:
                            nc.scalar.activation(eg[:], ps_s[:], AF.Exp)
                        else:
                            nc.scalar.activation(
                                eg[:].rearrange("p (j q) -> p j q",
                                                q=512)[:, :, o:512],
                                ps_s[:].rearrange("p (j q) -> p j q",
                                                  q=512)[:, :, o:512],
                                AF.Exp)
                            # causal triangle at the diagonal 128-col block
                            nc.gpsimd.affine_select(
                                eg[:].rearrange("p (j q) -> p j q",
                                                q=512)[:, :, o:o + 128],
                                eg[:].rearrange("p (j q) -> p j q",
                                                q=512)[:, :, o:o + 128],
                                pattern=[[0, 2], [1, 128]],
                                compare_op=OP.is_ge,
                                fill=0.0,
                                base=0,
                                channel_multiplier=-1)
                        flush_held()
                        is_last = kt == nkt - 1
                        held[0] = (
                            lambda kt=kt, eg=eg, o=o, f=emit_pv: f(kt, eg, o),
                            normalize if is_last else None)
                        # ~107ns LDWEIGHTS pulse every few k-tiles keeps the
                        # HAM activity window non-idle (no PSUM needed), so
                        # exp-bound stretches can't demote the PE to 1.2 GHz
                        if kt % (2 if qc < 2 else 3) == 1:
                            nc.tensor.ldweights(ident[:])
                # queue this q-chunk's out-projection groups (flushed during
                # qc+1; the final chunk's groups are flushed below)
                for tl in range(4):
                    for og in range(2):
                        pending.append((4 * qc + tl, og))
            flush_held()
            while pending:
                emit_ogroup()

    nc.compile()
    return nc


def _rope_tables(pos, norm_w, scale):
    """Build [P, NT, 4, 32] tables A,B,C,D for out1 = t1*A - t2*B,
    out2 = t1*C + t2*D (NeoX rope with folded norm weight + score scale)."""
    n_tok = pos.shape[0]
    f = np.arange(0, D_HEAD, 2, dtype=np.float64) / D_HEAD
    inv_freq = 1.0 / (ROPE_BASE ** f)                       # [32]
    ang = pos.astype(np.float64)[:, None] * inv_freq[None, :]  # [n, 32]
    cos, sin = np.cos(ang), np.sin(ang)
    w1 = norm_w[:32].astype(np.float64)
    w2 = norm_w[32:].astype(np.float64)
    A = cos * w1 * scale
    Bt = sin * w2 * scale
    C = sin * w1 * scale
    D = cos * w2 * scale
    # D negated: the kernel computes t1*(A,C) - t2*(B,-D) in two fused ops
    tab = np.stack([A, Bt, C, -D], axis=1).astype(np.float32)  # [n, 4, 32]
    return np.ascontiguousarray(
        tab.reshape(n_tok // P, P, 4, 32).transpose(1, 0, 2, 3))


def make_in_maps(x, pos, qkv_w, out_w, q_norm_w, k_norm_w, n_tok=N):
    import ml_dtypes
    bf16 = ml_dtypes.bfloat16

    scale = D_HEAD ** -0.5
    tabq = _rope_tables(pos, q_norm_w, scale).astype(bf16)
    tabk = _rope_tables(pos, k_norm_w, 1.0).astype(bf16)
    wq_all = qkv_w[0:H_Q * D_HEAD].reshape(H_Q, D_HEAD, D_MODEL)
    wk_all = qkv_w[H_Q * D_HEAD:(H_Q + H_KV) * D_HEAD].reshape(
        H_KV, D_HEAD, D_MODEL)
    wv_all = qkv_w[(H_Q + H_KV) * D_HEAD:].reshape(H_KV, D_HEAD, D_MODEL)
    wo_all = out_w.reshape(D_MODEL, H_Q, D_HEAD)

    in_maps = []
    for c in range(NCORES):
        b, hg = divmod(c, 4)
        heads = list(range(8 * hg, 8 * hg + 8))
        kvs = [2 * hg, 2 * hg + 1]
        wsel = np.concatenate([
            wq_all[heads].reshape(512, D_MODEL),
            wk_all[kvs].reshape(128, D_MODEL),
            wv_all[kvs].reshape(128, D_MODEL)], axis=0)    # [768, D]
        in_maps.append({
            "xT": np.ascontiguousarray(x[b].T).astype(bf16),
            "wqkv": np.ascontiguousarray(wsel.T).astype(bf16),
            "wo": np.ascontiguousarray(
                wo_all[:, heads].reshape(D_MODEL, 512).T).astype(bf16),
            "tabq": tabq,
            "tabk": tabk,
        })
    return in_maps


def _reference_host(x, mask, pos, qkv_w, out_w, q_norm_w, k_norm_w):
    """Pure-numpy fallback, used only if the mask is not causal."""
    xx = x.astype(np.float64)
    qkv = xx @ qkv_w.T.astype(np.float64)
    Bsz, Nl, _ = x.shape
    qkv = qkv.reshape(Bsz, Nl, H_Q + 2 * H_KV, D_HEAD).transpose(0, 2, 1, 3)
    q, k, v = (qkv[:, :H_Q], qkv[:, H_Q:H_Q + H_KV], qkv[:, H_Q + H_KV:])

    def rms(t, w):
        var = np.mean(t * t, axis=-1, keepdims=True)
        return t / np.sqrt(var + EPS) * w

    def rope(t):
        f = np.arange(0, D_HEAD, 2) / D_HEAD
        inv = 1.0 / (ROPE_BASE ** f)
        ang = pos.astype(np.float64)[:, None] * inv[None, :]
        cs, sn = np.cos(ang), np.sin(ang)
        t1, t2 = t[..., :32], t[..., 32:]
        return np.concatenate([t1 * cs - t2 * sn, t1 * sn + t2 * cs], axis=-1)

    q, k = rope(rms(q, q_norm_w)), rope(rms(k, k_norm_w))
    qg = q.reshape(Bsz, H_KV, 4, Nl, D_HEAD)
    sc = np.einsum("bhgnd,bhmd->bhgnm", qg, k) * (D_HEAD ** -0.5)
    sc = np.where(mask[None, None, None], -np.inf, sc)
    sc -= sc.max(axis=-1, keepdims=True)
    p = np.exp(sc)
    p /= p.sum(axis=-1, keepdims=True)
    y = np.einsum("bhgnm,bhmd->bhgnd", p, v)
    y = y.reshape(Bsz, H_Q, Nl, D_HEAD).transpose(0, 2, 1, 3).reshape(
        Bsz, Nl, D_MODEL)
    return (y @ out_w.T.astype(np.float64)).astype(np.float32)


_NC_CACHE = {}


def run_on_device(in_maps, n_tok=N, trace=False, trace_kwargs=None):
    import sys
    for p in ("/opt/trn_rl_repo",):
        if p not in sys.path:
            sys.path.insert(0, p)
    from concourse.bass_utils import run_bass_kernel_spmd

    key = n_tok
    if key not in _NC_CACHE:
        _NC_CACHE[key] = build_nc(n_tok)
    nc = _NC_CACHE[key]
    return run_bass_kernel_spmd(
        nc, in_maps, list(range(len(in_maps))), trace=trace,
        **(trace_kwargs or {}))


def kernel(x, mask, pos, qkv_w, out_w, q_norm_w, k_norm_w):
    x = np.asarray(x, dtype=np.float32)
    mask = np.asarray(mask)
    pos = np.asarray(pos)
    causal = bool(
        np.array_equal(mask,
                       np.triu(np.ones((N, N), dtype=bool), k=1)))
    if not causal:
        return _reference_host(x, mask, pos, np.asarray(qkv_w),
                               np.asarray(out_w), np.asarray(q_norm_w),
                               np.asarray(k_norm_w))
    in_maps = make_in_maps(x, pos, np.asarray(qkv_w, dtype=np.float32),
                           np.asarray(out_w, dtype=np.float32),
                           np.asarray(q_norm_w, dtype=np.float32),
                           np.asarray(k_norm_w, dtype=np.float32))
    res = run_on_device(in_maps)
    outs = [r["out"].astype(np.float32) for r in res.results]
    full = np.empty((B, N, D_MODEL), dtype=np.float32)
    for b in range(B):
        full[b] = outs[4 * b] + outs[4 * b + 1] + outs[4 * b + 2] + outs[4 * b + 3]
    return full


# revision 38
# speedup vs baseline: 1.0097x; 1.0083x over previous
"""GQA attention block (B=2, N=2048, D=2048, Hq=32, Hkv=8, d=64) on 8 TRN2 NeuronCores.

Sharding: core c = b*4 + hg  (data-parallel over batch b in {0,1}; tensor-parallel
over 4 head-groups hg, each owning 8 q-heads / 2 kv-heads).  Each core computes a
row-parallel partial of the output projection for its batch; the host sums the 4
partials per batch (fp16 partials).

All matmuls run in bf16 (fp32 matmul costs 4 PE cycles/row vs 1 for bf16);
PSUM accumulation stays fp32 and softmax exp reads fp32 PSUM scores.

Engine-balance notes:
 - ScalarE activation tables: phase 1 uses only {Square, Sqrt, Copy} (one
   sqrt_and_others set), phase 2 only {Exp, Copy} (one exp_and_others set) —
   avoids the ~1.3us per ACT_TABLE_LOAD ping-pong between Ln and Exp sets.
 - The two heads of a pair occupy disjoint 64-partition halves, so their K=64
   score matmuls auto-derive disjoint PE row-group tile_positions and run
   CONCURRENTLY (the pair's scoresT land side by side in one [128,1024] PSUM
   tile, one 1024-wide exp per k-tile).
 - PV runs as one 1024-wide bf16 matmul per k-tile (both heads share the kv
   head, V with an appended ones-column produces y plus the softmax
   denominator); 1/den via DVE reciprocal + GpSimd partition_broadcast (no
   PSUM bank, no broadcast matmul).
 - The out-projection is interleaved per 512-token q-chunk and shares the
   scores' PSUM ring; output DMA'd as fp16 partials.
"""

import numpy as np

D_MODEL = 2048
H_Q, H_KV, D_HEAD = 32, 8, 64
B = 2
N = 2048
ROPE_BASE = 10000.0
EPS = 1e-6
NCORES = 8
P = 128


def _modules():
    import sys

    for p in ("/opt/trn_rl_repo",):
        if p not in sys.path:
            sys.path.insert(0, p)
    import concourse.bass as bass
    import concourse.tile as tile
    from concourse import bacc, mybir
    from concourse.masks import make_identity

    return bass, tile, bacc, mybir, make_identity


def build_nc(n_tok=N, causal=True):
    """Build the single-core SPMD Bass program (identical on all 8 cores)."""
    from contextlib import ExitStack

    bass, tile, bacc, mybir, make_identity = _modules()
    f32 = mybir.dt.float32
    f16 = mybir.dt.float16
    bf16 = mybir.dt.bfloat16
    ts = bass.ts
    AF = mybir.ActivationFunctionType
    OP = mybir.AluOpType

    NT = n_tok // P           # token tiles
    DC = D_MODEL // P         # contraction chunks for qkv proj
    QC = n_tok // 512         # query chunks of 512
    NG = n_tok // 512         # x-load groups (512 tokens each)
    assert QC >= 1 and n_tok % 512 == 0

    nc = bacc.Bacc("TRN2", target_bir_lowering=False, debug=False,
                   num_devices=NCORES)

    xT = nc.dram_tensor("xT", [D_MODEL, n_tok], bf16, kind="ExternalInput").ap()
    wqkv = nc.dram_tensor("wqkv", [D_MODEL, 768], bf16, kind="ExternalInput").ap()
    wo = nc.dram_tensor("wo", [512, D_MODEL], bf16, kind="ExternalInput").ap()
    tabq = nc.dram_tensor("tabq", [P, NT, 4, 32], bf16, kind="ExternalInput").ap()
    tabk = nc.dram_tensor("tabk", [P, NT, 4, 32], bf16, kind="ExternalInput").ap()
    out = nc.dram_tensor("out", [n_tok, D_MODEL], f16, kind="ExternalOutput").ap()

    with ExitStack() as ctx:
        tc = ctx.enter_context(tile.TileContext(nc))

        cpool = ctx.enter_context(tc.tile_pool(name="const", bufs=1))
        # persistent activations (all bf16: they feed matmuls)
        qfm = [cpool.tile([P, n_tok], bf16, name=f"qfm{c}") for c in range(4)]
        kfm = cpool.tile([P, n_tok], bf16, name="kfm")     # [kv0 | kv1] on partitions
        kswap = cpool.tile([P, n_tok], bf16, name="kswap")  # [kv1 | kv0]
        yfm = [cpool.tile([P, n_tok], bf16, name=f"yfm{c}") for c in range(4)]
        vsb = [cpool.tile([P, 130], bf16, name=f"vsb{t}") for t in range(NT)]
        ident = cpool.tile([P, P], bf16, name="ident")
        make_identity(nc, ident[:])
        eps_t = cpool.tile([P, 1], f32, name="eps_t")
        nc.gpsimd.memset(eps_t[:], EPS)
        for t in range(NT):
            nc.gpsimd.memset(vsb[t][:, 64:65], 1.0)
            nc.gpsimd.memset(vsb[t][:, 129:130], 1.0)

        # ---------------- phase 1: qkv + norm + rope + transpose ----------
        with ExitStack() as p1:
            wpool = p1.enter_context(tc.tile_pool(name="wqkv", bufs=1))
            tpool = p1.enter_context(tc.tile_pool(name="tabs", bufs=1))
            xpool = p1.enter_context(tc.tile_pool(name="xg", bufs=3))
            # bufs=3: three token tiles in flight so each tile's serial
            # rmsnorm/rope/transpose chain overlaps the next tiles' matmuls
            wkk = p1.enter_context(tc.tile_pool(name="qkvwork", bufs=3))
            qkv_ps = p1.enter_context(
                tc.tile_pool(name="qkvpsum", bufs=2, space="PSUM"))
            # 4 bufs: the transpose->copy ring cycles ~0.5us per slot; with
            # only 2 the in-order tensor queue blocks at every 3rd transpose
            tp_ps = p1.enter_context(
                tc.tile_pool(name="tppsum", bufs=4, space="PSUM"))

            # x group 0 + first weight chunk issued first so the first qkv
            # matmul can start as early as possible (wq split into 4 tiles)
            xgs = {}
            xgs[0] = xpool.tile([P, DC, 256], bf16, tag="xg", name="xg0")
            nc.sync.dma_start(
                xgs[0][:], xT[:, ts(0, 256)].rearrange("(o p) t -> p o t", p=P))
            wq_sb = [wpool.tile([P, 4, 768], bf16, name=f"wq{i}")
                     for i in range(4)]
            wq_src = wqkv.rearrange("(o p) r -> p o r", p=P)
            for i in range(4):
                nc.sync.dma_start(wq_sb[i][:], wq_src[:, 4 * i:4 * i + 4, :])
            tq = tpool.tile([P, NT, 4, 32], bf16)
            nc.sync.dma_start(tq[:], tabq)
            tk = tpool.tile([P, NT, 4, 32], bf16)
            nc.sync.dma_start(tk[:], tabk)

            # PE warmup: dummy transposes into the tp ring promote the HAM
            # clock gate to 8/8 (2.4 GHz) and keep the PE busy while the
            # first x/w DMAs land (cold PE runs at 1.2 GHz; the activity
            # window is ~3.4us, so idling through the DMAs would re-demote).
            for wu in range(70):
                wt = tp_ps.tile([P, P], bf16, tag="tp")
                nc.tensor.transpose(wt[:], ident[:], ident[:])

            for g in range(2 * NG):
                if g not in xgs:
                    xgs[g] = xpool.tile([P, DC, 256], bf16, tag="xg",
                                        name=f"xg{g}")
                    nc.sync.dma_start(
                        xgs[g][:],
                        xT[:, ts(g, 256)].rearrange("(o p) t -> p o t", p=P))
                xg = xgs[g]
                for lt in range(2):
                    tt = g * 2 + lt
                    ps = qkv_ps.tile([P, 768], f32, tag="qkv")
                    for dc in range(DC):
                        lhsT = xg[:, dc, ts(lt, P)]
                        wsl = wq_sb[dc // 4][:, dc % 4]
                        nc.tensor.matmul(ps[:, 0:512], lhsT, wsl[:, 0:512],
                                         start=(dc == 0), stop=(dc == DC - 1))
                        nc.tensor.matmul(ps[:, 512:768], lhsT, wsl[:, 512:768],
                                         start=(dc == 0), stop=(dc == DC - 1))
                    # --- rmsnorm (Square+Sqrt stay in the sqrt table set) ---
                    sq = wkk.tile([P, 640], f32, tag="sq")
                    nc.scalar.activation(sq[:], ps[:, 0:640], AF.Square)
                    ssq = wkk.tile([P, 10], f32, tag="ssq")
                    nc.vector.reduce_sum(
                        ssq[:], sq[:].rearrange("p (h d) -> p h d", d=64),
                        axis=mybir.AxisListType.X)
                    sd = wkk.tile([P, 10], f32, tag="sd")
                    nc.scalar.activation(sd[:], ssq[:], AF.Sqrt,
                                         bias=eps_t[:], scale=1.0 / 64)
                    rs = wkk.tile([P, 10], f32, tag="rs")
                    nc.vector.reciprocal(rs[:], sd[:])
                    qn = wkk.tile([P, 512], bf16, tag="qn")
                    nc.vector.tensor_tensor(
                        qn[:].rearrange("p (h d) -> p h d", d=64),
                        ps[:, 0:512].rearrange("p (h d) -> p h d", d=64),
                        rs[:, 0:8, None].to_broadcast([P, 8, 64]), OP.mult)
                    kn = wkk.tile([P, 128], bf16, tag="kn")
                    nc.vector.tensor_tensor(
                        kn[:].rearrange("p (h d) -> p h d", d=64),
                        ps[:, 512:640].rearrange("p (h d) -> p h d", d=64),
                        rs[:, 8:10, None].to_broadcast([P, 2, 64]), OP.mult)
                    # --- v copy (ones cols at 64/129; one strided ACT) ---
                    nc.scalar.activation(
                        vsb[tt][:, 0:130].rearrange(
                            "p (j q) -> p j q", q=65)[:, :, 0:64],
                        ps[:, 640:768].rearrange("p (j q) -> p j q", q=64),
                        AF.Copy)
                    # --- rope: 3 DVE ops per tensor via host-folded tables
                    # tab rows are [A, B, C, -D]; viewed as [P, 2, 2, 32] the
                    # pairs are (A,C) and (B,-D), so
                    # dv = t1*(A,C) - t2*(B,-D) = (t1*A - t2*B | t1*C + t2*D)
                    qr = wkk.tile([P, 512], bf16, tag="qr")
                    kr = wkk.tile([P, 128], bf16, tag="kr")
                    for (src, dst, tab, nh) in ((qn, qr, tq, 8), (kn, kr, tk, 2)):
                        sv = src[:].rearrange("p (h d) -> p h d", d=64)
                        dv = dst[:].rearrange("p (h two f) -> p h two f",
                                              two=2, f=32)
                        tabv = tab[:, tt].rearrange("p (g two) f -> p two g f",
                                                    two=2)
                        t1 = sv[:, :, None, 0:32].to_broadcast([P, nh, 2, 32])
                        t2 = sv[:, :, None, 32:64].to_broadcast([P, nh, 2, 32])
                        AC = tabv[:, 0:1, :, :].to_broadcast([P, nh, 2, 32])
                        BD = tabv[:, 1:2, :, :].to_broadcast([P, nh, 2, 32])
                        u13 = wkk.tile([P, nh, 2, 32], bf16, tag=f"u13_{nh}")
                        u24 = wkk.tile([P, nh, 2, 32], bf16, tag=f"u24_{nh}")
                        nc.vector.tensor_tensor(u13[:], t1, AC, OP.mult)
                        nc.vector.tensor_tensor(u24[:], t2, BD, OP.mult)
                        nc.vector.tensor_tensor(dv, u13[:], u24[:],
                                                OP.subtract)
                    # --- transpose to feature-major (copies on ScalarE) ---
                    for rc in range(4):
                        pt = tp_ps.tile([P, P], bf16, tag="tp")
                        nc.tensor.transpose(pt[:], qr[:, ts(rc, P)], ident[:])
                        nc.scalar.activation(qfm[rc][:, ts(tt, P)], pt[:],
                                             AF.Copy)
                    pt = tp_ps.tile([P, P], bf16, tag="tp")
                    nc.tensor.transpose(pt[:], kr[:], ident[:])
                    nc.scalar.activation(kfm[:, ts(tt, P)], pt[:], AF.Copy)
                    # kswap: partition halves exchanged, built per tile
                    nc.scalar.activation(kswap[64:128, ts(tt, P)], pt[0:64, :],
                                         AF.Copy)
                    nc.scalar.activation(kswap[0:64, ts(tt, P)], pt[64:128, :],
                                         AF.Copy)

        # ---------------- phase 2: attention + out projection ------------
        wopool = ctx.enter_context(tc.tile_pool(name="wo", bufs=1))
        wo_sb = wopool.tile([P, 4, D_MODEL], bf16, name="wo_sb")
        nc.sync.dma_start(wo_sb[:], wo.rearrange("(o p) d -> p o d", p=P))
        with ExitStack() as p2:
            epool = p2.enter_context(tc.tile_pool(name="exp", bufs=4))
            npool = p2.enter_context(tc.tile_pool(name="nrm", bufs=2))
            opool = p2.enter_context(tc.tile_pool(name="osb", bufs=3))
            s_ps = p2.enter_context(
                tc.tile_pool(name="spsum", bufs=2, space="PSUM"))
            y_ps = p2.enter_context(
                tc.tile_pool(name="ypsum", bufs=2, space="PSUM"))

            # out-projection emission: groups for q-chunk qc are spread
            # through qc+1's attention stream so their matmuls fill the
            # tensor-engine gaps of the exp-bound kt pipeline (in-order
            # engine queues: the filler must sit between the stalls).
            pending = []          # (token tile, output half) groups
            emit_ctr = [0]

            def emit_ogroup():
                t, og = pending.pop(0)
                ps_o = s_ps.tile([P, 1024], f32, tag="s")
                for oc2 in range(2):
                    for yc in range(4):
                        nc.tensor.matmul(
                            ps_o[:, ts(oc2, 512)],
                            yfm[yc][:, ts(t, P)],
                            wo_sb[:, yc, 1024 * og + 512 * oc2:
                                  1024 * og + 512 * (oc2 + 1)],
                            start=(yc == 0), stop=(yc == 3))
                ob = opool.tile([P, 1024], f16, tag="ob")
                nc.vector.tensor_copy(ob[:], ps_o[:])
                nc.sync.dma_start(out[ts(t, P), ts(og, 1024)], ob[:])

            # `held` carries the not-yet-emitted PV (+ pair finalizer) of the
            # previous k-tile ACROSS pair boundaries, so scores/exp of the
            # next pair keep both engines fed while the last PV of the
            # previous pair waits on its exp semaphore.
            held = [None]   # (pv_fn, final_fn or None)

            def flush_held():
                if held[0] is None:
                    return
                pv_fn, final_fn = held[0]
                held[0] = None
                pv_fn()
                if final_fn is not None:
                    final_fn()
                emit_ctr[0] += 1
                if pending and emit_ctr[0] % 3 == 2:
                    emit_ogroup()

            for qc in range(QC):
                for c in range(4):
                    kv = c // 2
                    # A = head 2c (partitions 0:64), B = head 2c+1 (64:128)
                    ksA = kfm if kv == 0 else kswap
                    ksB = kswap if kv == 0 else kfm
                    vsl = slice(65 * kv, 65 * kv + 65)
                    nkt = 4 * qc + 4 if causal else 4 * QC
                    ps_y = y_ps.tile([65, 1024], f32, tag="y")

                    def emit_pv(kt, eg, o, ps_y=ps_y, vsl=vsl, nkt=nkt):
                        nc.tensor.matmul(
                            ps_y[:, o:512], vsb[kt][:, vsl], eg[:, o:512],
                            start=(kt == 0), stop=(kt == nkt - 1))
                        nc.tensor.matmul(
                            ps_y[:, 512 + o:1024], vsb[kt][:, vsl],
                            eg[:, 512 + o:1024],
                            start=(kt == 0), stop=(kt == nkt - 1))

                    def normalize(ps_y=ps_y, c=c, qc=qc):
                        # 1/den via DVE recip + GpSimd partition broadcast
                        # (recip can't read PSUM; vector copy bounces row 64)
                        draw = npool.tile([1, 1024], f32, tag="draw")
                        nc.vector.tensor_copy(draw[0:1, :], ps_y[64:65, :])
                        rec = npool.tile([1, 1024], f32, tag="rec")
                        nc.vector.reciprocal_approx_fast(rec[0:1, :],
                                                         draw[0:1, :])
                        rexp = npool.tile([64, 1024], f32, tag="rexp")
                        nc.gpsimd.partition_broadcast(rexp[:], rec[0:1, :],
                                                      channels=64)
                        nc.vector.tensor_tensor(yfm[c][0:64, ts(qc, 512)],
                                                ps_y[0:64, 0:512],
                                                rexp[:, 0:512], OP.mult)
                        nc.vector.tensor_tensor(yfm[c][64:128, ts(qc, 512)],
                                                ps_y[0:64, 512:1024],
                                                rexp[:, 512:1024], OP.mult)

                    for kt in range(nkt):
                        jl = kt - 4 * qc  # >=0 inside the diagonal quad
                        diag = causal and jl >= 0
                        o = 128 * jl if diag else 0
                        ps_s = s_ps.tile([P, 1024], f32, tag="s")
                        eg = epool.tile([P, 1024], bf16, tag="eg")
                        nc.tensor.matmul(
                            ps_s[:, o:512],
                            ksA[0:64, ts(kt, P)],
                            qfm[c][0:64, 512 * qc + o:512 * (qc + 1)],
                            start=True, stop=True)
                        nc.tensor.matmul(
                            ps_s[:, 512 + o:1024],
                            ksB[64:128, ts(kt, P)],
                            qfm[c][64:128, 512 * qc + o:512 * (qc + 1)],
                            start=True, stop=True)
                        if not diag:
                            nc.scalar.activation(eg[:], ps_s[:], AF.Exp)
                        else:
                            nc.scalar.activation(
                                eg[:].rearrange("p (j q) -> p j q",
                                                q=512)[:, :, o:512],
                                ps_s[:].rearrange("p (j q) -> p j q",
                                                  q=512)[:, :, o:512],
                                AF.Exp)
                            # causal triangle at the diagonal 128-col block
                            nc.gpsimd.affine_select(
                                eg[:].rearrange("p (j q) -> p j q",
                                                q=512)[:, :, o:o + 128],
                                eg[:].rearrange("p (j q) -> p j q",
                                                q=512)[:, :, o:o + 128],
                                pattern=[[0, 2], [1, 128]],
                                compare_op=OP.is_ge,
                                fill=0.0,
                                base=0,
                                channel_multiplier=-1)
                        flush_held()
                        is_last = kt == nkt - 1
                        held[0] = (
                            lambda kt=kt, eg=eg, o=o, f=emit_pv: f(kt, eg, o),
                            normalize if is_last else None)
                # queue this q-chunk's out-projection groups (flushed during
                # qc+1; the final chunk's groups are flushed below)
                for tl in range(4):
                    for og in range(2):
                        pending.append((4 * qc + tl, og))
            flush_held()
            while pending:
                emit_ogroup()

    nc.compile()
    return nc


def _rope_tables(pos, norm_w, scale):
    """Build [P, NT, 4, 32] tables A,B,C,D for out1 = t1*A - t2*B,
    out2 = t1*C + t2*D (NeoX rope with folded norm weight + score scale)."""
    n_tok = pos.shape[0]
    f = np.arange(0, D_HEAD, 2, dtype=np.float64) / D_HEAD
    inv_freq = 1.0 / (ROPE_BASE ** f)                       # [32]
    ang = pos.astype(np.float64)[:, None] * inv_freq[None, :]  # [n, 32]
    cos, sin = np.cos(ang), np.sin(ang)
    w1 = norm_w[:32].astype(np.float64)
    w2 = norm_w[32:].astype(np.float64)
    A = cos * w1 * scale
    Bt = sin * w2 * scale
    C = sin * w1 * scale
    D = cos * w2 * scale
    # D negated: the kernel computes t1*(A,C) - t2*(B,-D) in two fused ops
    tab = np.stack([A, Bt, C, -D], axis=1).astype(np.float32)  # [n, 4, 32]
    return np.ascontiguousarray(
        tab.reshape(n_tok // P, P, 4, 32).transpose(1, 0, 2, 3))


def make_in_maps(x, pos, qkv_w, out_w, q_norm_w, k_norm_w, n_tok=N):
    import ml_dtypes
    bf16 = ml_dtypes.bfloat16

    scale = D_HEAD ** -0.5
    tabq = _rope_tables(pos, q_norm_w, scale).astype(bf16)
    tabk = _rope_tables(pos, k_norm_w, 1.0).astype(bf16)
    wq_all = qkv_w[0:H_Q * D_HEAD].reshape(H_Q, D_HEAD, D_MODEL)
    wk_all = qkv_w[H_Q * D_HEAD:(H_Q + H_KV) * D_HEAD].reshape(
        H_KV, D_HEAD, D_MODEL)
    wv_all = qkv_w[(H_Q + H_KV) * D_HEAD:].reshape(H_KV, D_HEAD, D_MODEL)
    wo_all = out_w.reshape(D_MODEL, H_Q, D_HEAD)

    in_maps = []
    for c in range(NCORES):
        b, hg = divmod(c, 4)
        heads = list(range(8 * hg, 8 * hg + 8))
        kvs = [2 * hg, 2 * hg + 1]
        wsel = np.concatenate([
            wq_all[heads].reshape(512, D_MODEL),
            wk_all[kvs].reshape(128, D_MODEL),
            wv_all[kvs].reshape(128, D_MODEL)], axis=0)    # [768, D]
        in_maps.append({
            "xT": np.ascontiguousarray(x[b].T).astype(bf16),
            "wqkv": np.ascontiguousarray(wsel.T).astype(bf16),
            "wo": np.ascontiguousarray(
                wo_all[:, heads].reshape(D_MODEL, 512).T).astype(bf16),
            "tabq": tabq,
            "tabk": tabk,
        })
    return in_maps


def _reference_host(x, mask, pos, qkv_w, out_w, q_norm_w, k_norm_w):
    """Pure-numpy fallback, used only if the mask is not causal."""
    xx = x.astype(np.float64)
    qkv = xx @ qkv_w.T.astype(np.float64)
    Bsz, Nl, _ = x.shape
    qkv = qkv.reshape(Bsz, Nl, H_Q + 2 * H_KV, D_HEAD).transpose(0, 2, 1, 3)
    q, k, v = (qkv[:, :H_Q], qkv[:, H_Q:H_Q + H_KV], qkv[:, H_Q + H_KV:])

    def rms(t, w):
        var = np.mean(t * t, axis=-1, keepdims=True)
        return t / np.sqrt(var + EPS) * w

    def rope(t):
        f = np.arange(0, D_HEAD, 2) / D_HEAD
        inv = 1.0 / (ROPE_BASE ** f)
        ang = pos.astype(np.float64)[:, None] * inv[None, :]
        cs, sn = np.cos(ang), np.sin(ang)
        t1, t2 = t[..., :32], t[..., 32:]
        return np.concatenate([t1 * cs - t2 * sn, t1 * sn + t2 * cs], axis=-1)

    q, k = rope(rms(q, q_norm_w)), rope(rms(k, k_norm_w))
    qg = q.reshape(Bsz, H_KV, 4, Nl, D_HEAD)
    sc = np.einsum("bhgnd,bhmd->bhgnm", qg, k) * (D_HEAD ** -0.5)
    sc = np.where(mask[None, None, None], -np.inf, sc)
    sc -= sc.max(axis=-1, keepdims=True)
    p = np.exp(sc)
    p /= p.sum(axis=-1, keepdims=True)
    y = np.einsum("bhgnm,bhmd->bhgnd", p, v)
    y = y.reshape(Bsz, H_Q, Nl, D_HEAD).transpose(0, 2, 1, 3).reshape(
        Bsz, Nl, D_MODEL)
    return (y @ out_w.T.astype(np.float64)).astype(np.float32)


_NC_CACHE = {}


def run_on_device(in_maps, n_tok=N, trace=False, trace_kwargs=None):
    import sys
    for p in ("/opt/trn_rl_repo",):
        if p not in sys.path:
            sys.path.insert(0, p)
    from concourse.bass_utils import run_bass_kernel_spmd

    key = n_tok
    if key not in _NC_CACHE:
        _NC_CACHE[key] = build_nc(n_tok)
    nc = _NC_CACHE[key]
    return run_bass_kernel_spmd(
        nc, in_maps, list(range(len(in_maps))), trace=trace,
        **(trace_kwargs or {}))


def kernel(x, mask, pos, qkv_w, out_w, q_norm_w, k_norm_w):
    x = np.asarray(x, dtype=np.float32)
    mask = np.asarray(mask)
    pos = np.asarray(pos)
    causal = bool(
        np.array_equal(mask,
                       np.triu(np.ones((N, N), dtype=bool), k=1)))
    if not causal:
        return _reference_host(x, mask, pos, np.asarray(qkv_w),
                               np.asarray(out_w), np.asarray(q_norm_w),
                               np.asarray(k_norm_w))
    in_maps = make_in_maps(x, pos, np.asarray(qkv_w, dtype=np.float32),
                           np.asarray(out_w, dtype=np.float32),
                           np.asarray(q_norm_w, dtype=np.float32),
                           np.asarray(k_norm_w, dtype=np.float32))
    res = run_on_device(in_maps)
    outs = [r["out"].astype(np.float32) for r in res.results]
    full = np.empty((B, N, D_MODEL), dtype=np.float32)
    for b in range(B):
        full[b] = outs[4 * b] + outs[4 * b + 1] + outs[4 * b + 2] + outs[4 * b + 3]
    return full


# revision 39
# speedup vs baseline: 1.0141x; 1.0044x over previous
"""GQA attention block (B=2, N=2048, D=2048, Hq=32, Hkv=8, d=64) on 8 TRN2 NeuronCores.

Sharding: core c = b*4 + hg  (data-parallel over batch b in {0,1}; tensor-parallel
over 4 head-groups hg, each owning 8 q-heads / 2 kv-heads).  Each core computes a
row-parallel partial of the output projection for its batch; the host sums the 4
partials per batch (fp16 partials).

All matmuls run in bf16 (fp32 matmul costs 4 PE cycles/row vs 1 for bf16);
PSUM accumulation stays fp32 and softmax exp reads fp32 PSUM scores.

Engine-balance notes:
 - ScalarE activation tables: phase 1 uses only {Square, Sqrt, Copy} (one
   sqrt_and_others set), phase 2 only {Exp, Copy} (one exp_and_others set) —
   avoids the ~1.3us per ACT_TABLE_LOAD ping-pong between Ln and Exp sets.
 - The two heads of a pair occupy disjoint 64-partition halves, so their K=64
   score matmuls auto-derive disjoint PE row-group tile_positions and run
   CONCURRENTLY (the pair's scoresT land side by side in one [128,1024] PSUM
   tile, one 1024-wide exp per k-tile).
 - PV runs as one 1024-wide bf16 matmul per k-tile (both heads share the kv
   head, V with an appended ones-column produces y plus the softmax
   denominator); 1/den via DVE reciprocal + GpSimd partition_broadcast (no
   PSUM bank, no broadcast matmul).
 - The out-projection is interleaved per 512-token q-chunk and shares the
   scores' PSUM ring; output DMA'd as fp16 partials.
"""

import numpy as np

D_MODEL = 2048
H_Q, H_KV, D_HEAD = 32, 8, 64
B = 2
N = 2048
ROPE_BASE = 10000.0
EPS = 1e-6
NCORES = 8
P = 128


def _modules():
    import sys

    for p in ("/opt/trn_rl_repo",):
        if p not in sys.path:
            sys.path.insert(0, p)
    import concourse.bass as bass
    import concourse.tile as tile
    from concourse import bacc, mybir
    from concourse.masks import make_identity

    return bass, tile, bacc, mybir, make_identity


def build_nc(n_tok=N, causal=True):
    """Build the single-core SPMD Bass program (identical on all 8 cores)."""
    from contextlib import ExitStack

    bass, tile, bacc, mybir, make_identity = _modules()
    f32 = mybir.dt.float32
    f16 = mybir.dt.float16
    bf16 = mybir.dt.bfloat16
    ts = bass.ts
    AF = mybir.ActivationFunctionType
    OP = mybir.AluOpType

    NT = n_tok // P           # token tiles
    DC = D_MODEL // P         # contraction chunks for qkv proj
    QC = n_tok // 512         # query chunks of 512
    NG = n_tok // 512         # x-load groups (512 tokens each)
    assert QC >= 1 and n_tok % 512 == 0

    nc = bacc.Bacc("TRN2", target_bir_lowering=False, debug=False,
                   num_devices=NCORES)

    xT = nc.dram_tensor("xT", [D_MODEL, n_tok], bf16, kind="ExternalInput").ap()
    wqkv = nc.dram_tensor("wqkv", [D_MODEL, 768], bf16, kind="ExternalInput").ap()
    wo = nc.dram_tensor("wo", [512, D_MODEL], bf16, kind="ExternalInput").ap()
    tabq = nc.dram_tensor("tabq", [P, NT, 4, 32], bf16, kind="ExternalInput").ap()
    tabk = nc.dram_tensor("tabk", [P, NT, 4, 32], bf16, kind="ExternalInput").ap()
    out = nc.dram_tensor("out", [n_tok, D_MODEL], f16, kind="ExternalOutput").ap()

    with ExitStack() as ctx:
        tc = ctx.enter_context(tile.TileContext(nc))

        cpool = ctx.enter_context(tc.tile_pool(name="const", bufs=1))
        # persistent activations (all bf16: they feed matmuls)
        qfm = [cpool.tile([P, n_tok], bf16, name=f"qfm{c}") for c in range(4)]
        kfm = cpool.tile([P, n_tok], bf16, name="kfm")     # [kv0 | kv1] on partitions
        kswap = cpool.tile([P, n_tok], bf16, name="kswap")  # [kv1 | kv0]
        yfm = [cpool.tile([P, n_tok], bf16, name=f"yfm{c}") for c in range(4)]
        vsb = [cpool.tile([P, 130], bf16, name=f"vsb{t}") for t in range(NT)]
        ident = cpool.tile([P, P], bf16, name="ident")
        make_identity(nc, ident[:])
        eps_t = cpool.tile([P, 1], f32, name="eps_t")
        nc.gpsimd.memset(eps_t[:], EPS)
        for t in range(NT):
            nc.gpsimd.memset(vsb[t][:, 64:65], 1.0)
            nc.gpsimd.memset(vsb[t][:, 129:130], 1.0)

        # ---------------- phase 1: qkv + norm + rope + transpose ----------
        with ExitStack() as p1:
            wpool = p1.enter_context(tc.tile_pool(name="wqkv", bufs=1))
            tpool = p1.enter_context(tc.tile_pool(name="tabs", bufs=1))
            xpool = p1.enter_context(tc.tile_pool(name="xg", bufs=3))
            # bufs=3: three token tiles in flight so each tile's serial
            # rmsnorm/rope/transpose chain overlaps the next tiles' matmuls
            wkk = p1.enter_context(tc.tile_pool(name="qkvwork", bufs=3))
            qkv_ps = p1.enter_context(
                tc.tile_pool(name="qkvpsum", bufs=2, space="PSUM"))
            # 4 bufs: the transpose->copy ring cycles ~0.5us per slot; with
            # only 2 the in-order tensor queue blocks at every 3rd transpose
            tp_ps = p1.enter_context(
                tc.tile_pool(name="tppsum", bufs=4, space="PSUM"))

            # x group 0 + first weight chunk issued first so the first qkv
            # matmul can start as early as possible (wq split into 4 tiles)
            xgs = {}
            xgs[0] = xpool.tile([P, DC, 256], bf16, tag="xg", name="xg0")
            nc.sync.dma_start(
                xgs[0][:], xT[:, ts(0, 256)].rearrange("(o p) t -> p o t", p=P))
            wq_sb = [wpool.tile([P, 4, 768], bf16, name=f"wq{i}")
                     for i in range(4)]
            wq_src = wqkv.rearrange("(o p) r -> p o r", p=P)
            for i in range(4):
                nc.sync.dma_start(wq_sb[i][:], wq_src[:, 4 * i:4 * i + 4, :])
            tq = tpool.tile([P, NT, 4, 32], bf16)
            nc.sync.dma_start(tq[:], tabq)
            tk = tpool.tile([P, NT, 4, 32], bf16)
            nc.sync.dma_start(tk[:], tabk)

            # PE warmup: dummy transposes into the tp ring promote the HAM
            # clock gate to 8/8 (2.4 GHz) and keep the PE busy while the
            # first x/w DMAs land (cold PE runs at 1.2 GHz; the activity
            # window is ~3.4us, so idling through the DMAs would re-demote).
            for wu in range(70):
                wt = tp_ps.tile([P, P], bf16, tag="tp")
                nc.tensor.transpose(wt[:], ident[:], ident[:])

            for g in range(2 * NG):
                if g not in xgs:
                    xgs[g] = xpool.tile([P, DC, 256], bf16, tag="xg",
                                        name=f"xg{g}")
                    nc.sync.dma_start(
                        xgs[g][:],
                        xT[:, ts(g, 256)].rearrange("(o p) t -> p o t", p=P))
                xg = xgs[g]
                for lt in range(2):
                    tt = g * 2 + lt
                    ps = qkv_ps.tile([P, 768], f32, tag="qkv")
                    for dc in range(DC):
                        lhsT = xg[:, dc, ts(lt, P)]
                        wsl = wq_sb[dc // 4][:, dc % 4]
                        nc.tensor.matmul(ps[:, 0:512], lhsT, wsl[:, 0:512],
                                         start=(dc == 0), stop=(dc == DC - 1))
                        nc.tensor.matmul(ps[:, 512:768], lhsT, wsl[:, 512:768],
                                         start=(dc == 0), stop=(dc == DC - 1))
                    # --- rmsnorm (Square+Sqrt stay in the sqrt table set) ---
                    sq = wkk.tile([P, 640], f32, tag="sq")
                    nc.scalar.activation(sq[:], ps[:, 0:640], AF.Square)
                    ssq = wkk.tile([P, 10], f32, tag="ssq")
                    nc.vector.reduce_sum(
                        ssq[:], sq[:].rearrange("p (h d) -> p h d", d=64),
                        axis=mybir.AxisListType.X)
                    sd = wkk.tile([P, 10], f32, tag="sd")
                    nc.scalar.activation(sd[:], ssq[:], AF.Sqrt,
                                         bias=eps_t[:], scale=1.0 / 64)
                    rs = wkk.tile([P, 10], f32, tag="rs")
                    nc.vector.reciprocal(rs[:], sd[:])
                    qn = wkk.tile([P, 512], bf16, tag="qn")
                    nc.vector.tensor_tensor(
                        qn[:].rearrange("p (h d) -> p h d", d=64),
                        ps[:, 0:512].rearrange("p (h d) -> p h d", d=64),
                        rs[:, 0:8, None].to_broadcast([P, 8, 64]), OP.mult)
                    kn = wkk.tile([P, 128], bf16, tag="kn")
                    nc.vector.tensor_tensor(
                        kn[:].rearrange("p (h d) -> p h d", d=64),
                        ps[:, 512:640].rearrange("p (h d) -> p h d", d=64),
                        rs[:, 8:10, None].to_broadcast([P, 2, 64]), OP.mult)
                    # --- v copy (ones cols at 64/129; one strided ACT) ---
                    nc.scalar.activation(
                        vsb[tt][:, 0:130].rearrange(
                            "p (j q) -> p j q", q=65)[:, :, 0:64],
                        ps[:, 640:768].rearrange("p (j q) -> p j q", q=64),
                        AF.Copy)
                    # --- rope: 3 DVE ops per tensor via host-folded tables
                    # tab rows are [A, B, C, -D]; viewed as [P, 2, 2, 32] the
                    # pairs are (A,C) and (B,-D), so
                    # dv = t1*(A,C) - t2*(B,-D) = (t1*A - t2*B | t1*C + t2*D)
                    qr = wkk.tile([P, 512], bf16, tag="qr")
                    kr = wkk.tile([P, 128], bf16, tag="kr")
                    for (src, dst, tab, nh) in ((qn, qr, tq, 8), (kn, kr, tk, 2)):
                        sv = src[:].rearrange("p (h d) -> p h d", d=64)
                        dv = dst[:].rearrange("p (h two f) -> p h two f",
                                              two=2, f=32)
                        tabv = tab[:, tt].rearrange("p (g two) f -> p two g f",
                                                    two=2)
                        t1 = sv[:, :, None, 0:32].to_broadcast([P, nh, 2, 32])
                        t2 = sv[:, :, None, 32:64].to_broadcast([P, nh, 2, 32])
                        AC = tabv[:, 0:1, :, :].to_broadcast([P, nh, 2, 32])
                        BD = tabv[:, 1:2, :, :].to_broadcast([P, nh, 2, 32])
                        u13 = wkk.tile([P, nh, 2, 32], bf16, tag=f"u13_{nh}")
                        u24 = wkk.tile([P, nh, 2, 32], bf16, tag=f"u24_{nh}")
                        nc.vector.tensor_tensor(u13[:], t1, AC, OP.mult)
                        nc.vector.tensor_tensor(u24[:], t2, BD, OP.mult)
                        nc.vector.tensor_tensor(dv, u13[:], u24[:],
                                                OP.subtract)
                    # --- transpose to feature-major (copies on ScalarE) ---
                    for rc in range(4):
                        pt = tp_ps.tile([P, P], bf16, tag="tp")
                        nc.tensor.transpose(pt[:], qr[:, ts(rc, P)], ident[:])
                        nc.scalar.activation(qfm[rc][:, ts(tt, P)], pt[:],
                                             AF.Copy)
                    pt = tp_ps.tile([P, P], bf16, tag="tp")
                    nc.tensor.transpose(pt[:], kr[:], ident[:])
                    nc.scalar.activation(kfm[:, ts(tt, P)], pt[:], AF.Copy)
                    # kswap: partition halves exchanged, built per tile
                    nc.scalar.activation(kswap[64:128, ts(tt, P)], pt[0:64, :],
                                         AF.Copy)
                    nc.scalar.activation(kswap[0:64, ts(tt, P)], pt[64:128, :],
                                         AF.Copy)
            # transition burst: after the last tile's copies drain, these
            # no-consumer transposes run back-to-back (~3.8us sustained PE
            # activity), re-promoting the HAM clock gate to 2.4 GHz before
            # phase 2's fragmented exp-bound stream begins — without this
            # the first ~35us of attention run at 1.2 GHz (promotion needs
            # a fully-busy 3.4us window that fragmented activity never has;
            # LDWEIGHTS pulses do not count as PE-array activity)
            for wu in range(48):
                wt = tp_ps.tile([P, P], bf16, tag="tp")
                nc.tensor.transpose(wt[:], ident[:], ident[:])

        # ---------------- phase 2: attention + out projection ------------
        wopool = ctx.enter_context(tc.tile_pool(name="wo", bufs=1))
        wo_sb = wopool.tile([P, 4, D_MODEL], bf16, name="wo_sb")
        nc.sync.dma_start(wo_sb[:], wo.rearrange("(o p) d -> p o d", p=P))
        with ExitStack() as p2:
            epool = p2.enter_context(tc.tile_pool(name="exp", bufs=4))
            npool = p2.enter_context(tc.tile_pool(name="nrm", bufs=2))
            opool = p2.enter_context(tc.tile_pool(name="osb", bufs=3))
            s_ps = p2.enter_context(
                tc.tile_pool(name="spsum", bufs=2, space="PSUM"))
            y_ps = p2.enter_context(
                tc.tile_pool(name="ypsum", bufs=2, space="PSUM"))

            # out-projection emission: groups for q-chunk qc are spread
            # through qc+1's attention stream so their matmuls fill the
            # tensor-engine gaps of the exp-bound kt pipeline (in-order
            # engine queues: the filler must sit between the stalls).
            pending = []          # (token tile, output half) groups
            emit_ctr = [0]

            def emit_ogroup():
                t, og = pending.pop(0)
                ps_o = s_ps.tile([P, 1024], f32, tag="s")
                for oc2 in range(2):
                    for yc in range(4):
                        nc.tensor.matmul(
                            ps_o[:, ts(oc2, 512)],
                            yfm[yc][:, ts(t, P)],
                            wo_sb[:, yc, 1024 * og + 512 * oc2:
                                  1024 * og + 512 * (oc2 + 1)],
                            start=(yc == 0), stop=(yc == 3))
                ob = opool.tile([P, 1024], f16, tag="ob")
                nc.vector.tensor_copy(ob[:], ps_o[:])
                nc.sync.dma_start(out[ts(t, P), ts(og, 1024)], ob[:])

            # `held` carries the not-yet-emitted PV (+ pair finalizer) of the
            # previous k-tile ACROSS pair boundaries, so scores/exp of the
            # next pair keep both engines fed while the last PV of the
            # previous pair waits on its exp semaphore.
            held = [None]   # (pv_fn, final_fn or None)

            def flush_held():
                if held[0] is None:
                    return
                pv_fn, final_fn = held[0]
                held[0] = None
                pv_fn()
                if final_fn is not None:
                    final_fn()
                emit_ctr[0] += 1
                if pending and emit_ctr[0] % 3 == 2:
                    emit_ogroup()

            for qc in range(QC):
                for c in range(4):
                    kv = c // 2
                    # A = head 2c (partitions 0:64), B = head 2c+1 (64:128)
                    ksA = kfm if kv == 0 else kswap
                    ksB = kswap if kv == 0 else kfm
                    vsl = slice(65 * kv, 65 * kv + 65)
                    nkt = 4 * qc + 4 if causal else 4 * QC
                    ps_y = y_ps.tile([65, 1024], f32, tag="y")

                    def emit_pv(kt, eg, o, ps_y=ps_y, vsl=vsl, nkt=nkt):
                        nc.tensor.matmul(
                            ps_y[:, o:512], vsb[kt][:, vsl], eg[:, o:512],
                            start=(kt == 0), stop=(kt == nkt - 1))
                        nc.tensor.matmul(
                            ps_y[:, 512 + o:1024], vsb[kt][:, vsl],
                            eg[:, 512 + o:1024],
                            start=(kt == 0), stop=(kt == nkt - 1))

                    def normalize(ps_y=ps_y, c=c, qc=qc):
                        # 1/den via DVE recip + GpSimd partition broadcast
                        # (recip can't read PSUM; vector copy bounces row 64)
                        draw = npool.tile([1, 1024], f32, tag="draw")
                        nc.vector.tensor_copy(draw[0:1, :], ps_y[64:65, :])
                        rec = npool.tile([1, 1024], f32, tag="rec")
                        nc.vector.reciprocal_approx_fast(rec[0:1, :],
                                                         draw[0:1, :])
                        rexp = npool.tile([64, 1024], f32, tag="rexp")
                        nc.gpsimd.partition_broadcast(rexp[:], rec[0:1, :],
                                                      channels=64)
                        nc.vector.tensor_tensor(yfm[c][0:64, ts(qc, 512)],
                                                ps_y[0:64, 0:512],
                                                rexp[:, 0:512], OP.mult)
                        nc.vector.tensor_tensor(yfm[c][64:128, ts(qc, 512)],
                                                ps_y[0:64, 512:1024],
                                                rexp[:, 512:1024], OP.mult)

                    for kt in range(nkt):
                        jl = kt - 4 * qc  # >=0 inside the diagonal quad
                        diag = causal and jl >= 0
                        o = 128 * jl if diag else 0
                        ps_s = s_ps.tile([P, 1024], f32, tag="s")
                        eg = epool.tile([P, 1024], bf16, tag="eg")
                        nc.tensor.matmul(
                            ps_s[:, o:512],
                            ksA[0:64, ts(kt, P)],
                            qfm[c][0:64, 512 * qc + o:512 * (qc + 1)],
                            start=True, stop=True)
                        nc.tensor.matmul(
                            ps_s[:, 512 + o:1024],
                            ksB[64:128, ts(kt, P)],
                            qfm[c][64:128, 512 * qc + o:512 * (qc + 1)],
                            start=True, stop=True)
                        if not diag:
                            nc.scalar.activation(eg[:], ps_s[:], AF.Exp)
                        else:
                            nc.scalar.activation(
                                eg[:].rearrange("p (j q) -> p j q",
                                                q=512)[:, :, o:512],
                                ps_s[:].rearrange("p (j q) -> p j q",
                                                  q=512)[:, :, o:512],
                                AF.Exp)
                            # causal triangle at the diagonal 128-col block
                            nc.gpsimd.affine_select(
                                eg[:].rearrange("p (j q) -> p j q",
                                                q=512)[:, :, o:o + 128],
                                eg[:].rearrange("p (j q) -> p j q",
                                                q=512)[:, :, o:o + 128],
                                pattern=[[0, 2], [1, 128]],
                                compare_op=OP.is_ge,
                                fill=0.0,
                                base=0,
                                channel_multiplier=-1)
                        flush_held()
                        is_last = kt == nkt - 1
                        held[0] = (
                            lambda kt=kt, eg=eg, o=o, f=emit_pv: f(kt, eg, o),
                            normalize if is_last else None)
                # queue this q-chunk's out-projection groups (flushed during
                # qc+1; the final chunk's groups are flushed below)
                for tl in range(4):
                    for og in range(2):
                        pending.append((4 * qc + tl, og))
            flush_held()
            while pending:
                emit_ogroup()

    nc.compile()
    return nc


def _rope_tables(pos, norm_w, scale):
    """Build [P, NT, 4, 32] tables A,B,C,D for out1 = t1*A - t2*B,
    out2 = t1*C + t2*D (NeoX rope with folded norm weight + score scale)."""
    n_tok = pos.shape[0]
    f = np.arange(0, D_HEAD, 2, dtype=np.float64) / D_HEAD
    inv_freq = 1.0 / (ROPE_BASE ** f)                       # [32]
    ang = pos.astype(np.float64)[:, None] * inv_freq[None, :]  # [n, 32]
    cos, sin = np.cos(ang), np.sin(ang)
    w1 = norm_w[:32].astype(np.float64)
    w2 = norm_w[32:].astype(np.float64)
    A = cos * w1 * scale
    Bt = sin * w2 * scale
    C = sin * w1 * scale
    D = cos * w2 * scale
    # D negated: the kernel computes t1*(A,C) - t2*(B,-D) in two fused ops
    tab = np.stack([A, Bt, C, -D], axis=1).astype(np.float32)  # [n, 4, 32]
    return np.ascontiguousarray(
        tab.reshape(n_tok // P, P, 4, 32).transpose(1, 0, 2, 3))


def make_in_maps(x, pos, qkv_w, out_w, q_norm_w, k_norm_w, n_tok=N):
    import ml_dtypes
    bf16 = ml_dtypes.bfloat16

    scale = D_HEAD ** -0.5
    tabq = _rope_tables(pos, q_norm_w, scale).astype(bf16)
    tabk = _rope_tables(pos, k_norm_w, 1.0).astype(bf16)
    wq_all = qkv_w[0:H_Q * D_HEAD].reshape(H_Q, D_HEAD, D_MODEL)
    wk_all = qkv_w[H_Q * D_HEAD:(H_Q + H_KV) * D_HEAD].reshape(
        H_KV, D_HEAD, D_MODEL)
    wv_all = qkv_w[(H_Q + H_KV) * D_HEAD:].reshape(H_KV, D_HEAD, D_MODEL)
    wo_all = out_w.reshape(D_MODEL, H_Q, D_HEAD)

    in_maps = []
    for c in range(NCORES):
        b, hg = divmod(c, 4)
        heads = list(range(8 * hg, 8 * hg + 8))
        kvs = [2 * hg, 2 * hg + 1]
        wsel = np.concatenate([
            wq_all[heads].reshape(512, D_MODEL),
            wk_all[kvs].reshape(128, D_MODEL),
            wv_all[kvs].reshape(128, D_MODEL)], axis=0)    # [768, D]
        in_maps.append({
            "xT": np.ascontiguousarray(x[b].T).astype(bf16),
            "wqkv": np.ascontiguousarray(wsel.T).astype(bf16),
            "wo": np.ascontiguousarray(
                wo_all[:, heads].reshape(D_MODEL, 512).T).astype(bf16),
            "tabq": tabq,
            "tabk": tabk,
        })
    return in_maps


def _reference_host(x, mask, pos, qkv_w, out_w, q_norm_w, k_norm_w):
    """Pure-numpy fallback, used only if the mask is not causal."""
    xx = x.astype(np.float64)
    qkv = xx @ qkv_w.T.astype(np.float64)
    Bsz, Nl, _ = x.shape
    qkv = qkv.reshape(Bsz, Nl, H_Q + 2 * H_KV, D_HEAD).transpose(0, 2, 1, 3)
    q, k, v = (qkv[:, :H_Q], qkv[:, H_Q:H_Q + H_KV], qkv[:, H_Q + H_KV:])

    def rms(t, w):
        var = np.mean(t * t, axis=-1, keepdims=True)
        return t / np.sqrt(var + EPS) * w

    def rope(t):
        f = np.arange(0, D_HEAD, 2) / D_HEAD
        inv = 1.0 / (ROPE_BASE ** f)
        ang = pos.astype(np.float64)[:, None] * inv[None, :]
        cs, sn = np.cos(ang), np.sin(ang)
        t1, t2 = t[..., :32], t[..., 32:]
        return np.concatenate([t1 * cs - t2 * sn, t1 * sn + t2 * cs], axis=-1)

    q, k = rope(rms(q, q_norm_w)), rope(rms(k, k_norm_w))
    qg = q.reshape(Bsz, H_KV, 4, Nl, D_HEAD)
    sc = np.einsum("bhgnd,bhmd->bhgnm", qg, k) * (D_HEAD ** -0.5)
    sc = np.where(mask[None, None, None], -np.inf, sc)
    sc -= sc.max(axis=-1, keepdims=True)
    p = np.exp(sc)
    p /= p.sum(axis=-1, keepdims=True)
    y = np.einsum("bhgnm,bhmd->bhgnd", p, v)
    y = y.reshape(Bsz, H_Q, Nl, D_HEAD).transpose(0, 2, 1, 3).reshape(
        Bsz, Nl, D_MODEL)
    return (y @ out_w.T.astype(np.float64)).astype(np.float32)


_NC_CACHE = {}


def run_on_device(in_maps, n_tok=N, trace=False, trace_kwargs=None):
    import sys
    for p in ("/opt/trn_rl_repo",):
        if p not in sys.path:
            sys.path.insert(0, p)
    from concourse.bass_utils import run_bass_kernel_spmd

    key = n_tok
    if key not in _NC_CACHE:
        _NC_CACHE[key] = build_nc(n_tok)
    nc = _NC_CACHE[key]
    return run_bass_kernel_spmd(
        nc, in_maps, list(range(len(in_maps))), trace=trace,
        **(trace_kwargs or {}))


def kernel(x, mask, pos, qkv_w, out_w, q_norm_w, k_norm_w):
    x = np.asarray(x, dtype=np.float32)
    mask = np.asarray(mask)
    pos = np.asarray(pos)
    causal = bool(
        np.array_equal(mask,
                       np.triu(np.ones((N, N), dtype=bool), k=1)))
    if not causal:
        return _reference_host(x, mask, pos, np.asarray(qkv_w),
                               np.asarray(out_w), np.asarray(q_norm_w),
                               np.asarray(k_norm_w))
    in_maps = make_in_maps(x, pos, np.asarray(qkv_w, dtype=np.float32),
                           np.asarray(out_w, dtype=np.float32),
                           np.asarray(q_norm_w, dtype=np.float32),
                           np.asarray(k_norm_w, dtype=np.float32))
    res = run_on_device(in_maps)
    outs = [r["out"].astype(np.float32) for r in res.results]
    full = np.empty((B, N, D_MODEL), dtype=np.float32)
    for b in range(B):
        full[b] = outs[4 * b] + outs[4 * b + 1] + outs[4 * b + 2] + outs[4 * b + 3]
    return full


# revision 42
# speedup vs baseline: 1.0357x; 1.0213x over previous
"""GQA attention block (B=2, N=2048, D=2048, Hq=32, Hkv=8, d=64) on 8 TRN2 NeuronCores.

Sharding: core c = b*4 + hg  (data-parallel over batch b in {0,1}; tensor-parallel
over 4 head-groups hg, each owning 8 q-heads / 2 kv-heads).  Each core computes a
row-parallel partial of the output projection for its batch; the host sums the 4
partials per batch (fp16 partials).

All matmuls run in bf16 (fp32 matmul costs 4 PE cycles/row vs 1 for bf16);
PSUM accumulation stays fp32 and softmax exp reads fp32 PSUM scores.

Engine-balance notes:
 - ScalarE activation tables: phase 1 uses only {Square, Sqrt, Copy} (one
   sqrt_and_others set), phase 2 only {Exp, Copy} (one exp_and_others set) —
   avoids the ~1.3us per ACT_TABLE_LOAD ping-pong between Ln and Exp sets.
 - The two heads of a pair occupy disjoint 64-partition halves, so their K=64
   score matmuls auto-derive disjoint PE row-group tile_positions and run
   CONCURRENTLY (the pair's scoresT land side by side in one [128,1024] PSUM
   tile, one 1024-wide exp per k-tile).
 - PV runs as one 1024-wide bf16 matmul per k-tile (both heads share the kv
   head, V with an appended ones-column produces y plus the softmax
   denominator); 1/den via DVE reciprocal + GpSimd partition_broadcast (no
   PSUM bank, no broadcast matmul).
 - The out-projection is interleaved per 512-token q-chunk and shares the
   scores' PSUM ring; output DMA'd as fp16 partials.
"""

import numpy as np

D_MODEL = 2048
H_Q, H_KV, D_HEAD = 32, 8, 64
B = 2
N = 2048
ROPE_BASE = 10000.0
EPS = 1e-6
NCORES = 8
P = 128


def _modules():
    import sys

    for p in ("/opt/trn_rl_repo",):
        if p not in sys.path:
            sys.path.insert(0, p)
    import concourse.bass as bass
    import concourse.tile as tile
    from concourse import bacc, mybir
    from concourse.masks import make_identity

    return bass, tile, bacc, mybir, make_identity


def build_nc(n_tok=N, causal=True):
    """Build the single-core SPMD Bass program (identical on all 8 cores)."""
    from contextlib import ExitStack

    bass, tile, bacc, mybir, make_identity = _modules()
    f32 = mybir.dt.float32
    f16 = mybir.dt.float16
    bf16 = mybir.dt.bfloat16
    ts = bass.ts
    AF = mybir.ActivationFunctionType
    OP = mybir.AluOpType

    NT = n_tok // P           # token tiles
    DC = D_MODEL // P         # contraction chunks for qkv proj
    QC = n_tok // 512         # query chunks of 512
    NG = n_tok // 512         # x-load groups (512 tokens each)
    assert QC >= 1 and n_tok % 512 == 0

    nc = bacc.Bacc("TRN2", target_bir_lowering=False, debug=False,
                   num_devices=NCORES)

    xT = nc.dram_tensor("xT", [D_MODEL, n_tok], bf16, kind="ExternalInput").ap()
    wqkv = nc.dram_tensor("wqkv", [D_MODEL, 768], bf16, kind="ExternalInput").ap()
    wo = nc.dram_tensor("wo", [512, D_MODEL], bf16, kind="ExternalInput").ap()
    tabq = nc.dram_tensor("tabq", [P, NT, 4, 32], bf16, kind="ExternalInput").ap()
    tabk = nc.dram_tensor("tabk", [P, NT, 4, 32], bf16, kind="ExternalInput").ap()
    out = nc.dram_tensor("out", [n_tok, D_MODEL], f16, kind="ExternalOutput").ap()

    with ExitStack() as ctx:
        tc = ctx.enter_context(tile.TileContext(nc))

        cpool = ctx.enter_context(tc.tile_pool(name="const", bufs=1))
        # persistent activations (all bf16: they feed matmuls)
        qfm = [cpool.tile([P, n_tok], bf16, name=f"qfm{c}") for c in range(4)]
        kfm = cpool.tile([P, n_tok], bf16, name="kfm")     # [kv0 | kv1] on partitions
        kswap = cpool.tile([P, n_tok], bf16, name="kswap")  # [kv1 | kv0]
        yfm = [cpool.tile([P, n_tok], bf16, name=f"yfm{c}") for c in range(4)]
        vsb = [cpool.tile([P, 130], bf16, name=f"vsb{t}") for t in range(NT)]
        ident = cpool.tile([P, P], bf16, name="ident")
        make_identity(nc, ident[:])
        eps_t = cpool.tile([P, 1], f32, name="eps_t")
        nc.gpsimd.memset(eps_t[:], EPS)
        for t in range(NT):
            nc.gpsimd.memset(vsb[t][:, 64:65], 1.0)
            nc.gpsimd.memset(vsb[t][:, 129:130], 1.0)

        # ---------------- phase 1: qkv + norm + rope + transpose ----------
        with ExitStack() as p1:
            wpool = p1.enter_context(tc.tile_pool(name="wqkv", bufs=1))
            tpool = p1.enter_context(tc.tile_pool(name="tabs", bufs=1))
            xpool = p1.enter_context(tc.tile_pool(name="xg", bufs=3))
            # bufs=3: three token tiles in flight so each tile's serial
            # rmsnorm/rope/transpose chain overlaps the next tiles' matmuls
            wkk = p1.enter_context(tc.tile_pool(name="qkvwork", bufs=3))
            qkv_ps = p1.enter_context(
                tc.tile_pool(name="qkvpsum", bufs=2, space="PSUM"))
            # 4 bufs: the transpose->copy ring cycles ~0.5us per slot; with
            # only 2 the in-order tensor queue blocks at every 3rd transpose
            tp_ps = p1.enter_context(
                tc.tile_pool(name="tppsum", bufs=4, space="PSUM"))

            # x group 0 + first weight chunk issued first so the first qkv
            # matmul can start as early as possible (wq split into 4 tiles)
            xgs = {}
            xgs[0] = xpool.tile([P, DC, 256], bf16, tag="xg", name="xg0")
            nc.sync.dma_start(
                xgs[0][:], xT[:, ts(0, 256)].rearrange("(o p) t -> p o t", p=P))
            wq_sb = [wpool.tile([P, 4, 768], bf16, name=f"wq{i}")
                     for i in range(4)]
            wq_src = wqkv.rearrange("(o p) r -> p o r", p=P)
            for i in range(4):
                nc.sync.dma_start(wq_sb[i][:], wq_src[:, 4 * i:4 * i + 4, :])
            tq = tpool.tile([P, NT, 4, 32], bf16)
            nc.sync.dma_start(tq[:], tabq)
            tk = tpool.tile([P, NT, 4, 32], bf16)
            nc.sync.dma_start(tk[:], tabk)

            # PE warmup: dummy transposes into the tp ring promote the HAM
            # clock gate to 8/8 (2.4 GHz) and keep the PE busy while the
            # first x/w DMAs land (cold PE runs at 1.2 GHz; the activity
            # window is ~3.4us, so idling through the DMAs would re-demote).
            for wu in range(70):
                wt = tp_ps.tile([P, P], bf16, tag="tp")
                nc.tensor.transpose(wt[:], ident[:], ident[:])

            for g in range(2 * NG):
                if g not in xgs:
                    xgs[g] = xpool.tile([P, DC, 256], bf16, tag="xg",
                                        name=f"xg{g}")
                    nc.sync.dma_start(
                        xgs[g][:],
                        xT[:, ts(g, 256)].rearrange("(o p) t -> p o t", p=P))
                xg = xgs[g]
                for lt in range(2):
                    tt = g * 2 + lt
                    ps = qkv_ps.tile([P, 768], f32, tag="qkv")
                    for dc in range(DC):
                        lhsT = xg[:, dc, ts(lt, P)]
                        wsl = wq_sb[dc // 4][:, dc % 4]
                        nc.tensor.matmul(ps[:, 0:512], lhsT, wsl[:, 0:512],
                                         start=(dc == 0), stop=(dc == DC - 1))
                        nc.tensor.matmul(ps[:, 512:768], lhsT, wsl[:, 512:768],
                                         start=(dc == 0), stop=(dc == DC - 1))
                    # --- rmsnorm (Square+Sqrt stay in the sqrt table set) ---
                    sq = wkk.tile([P, 640], f32, tag="sq")
                    nc.scalar.activation(sq[:], ps[:, 0:640], AF.Square)
                    ssq = wkk.tile([P, 10], f32, tag="ssq")
                    nc.vector.reduce_sum(
                        ssq[:], sq[:].rearrange("p (h d) -> p h d", d=64),
                        axis=mybir.AxisListType.X)
                    sd = wkk.tile([P, 10], f32, tag="sd")
                    nc.scalar.activation(sd[:], ssq[:], AF.Sqrt,
                                         bias=eps_t[:], scale=1.0 / 64)
                    rs = wkk.tile([P, 10], f32, tag="rs")
                    nc.vector.reciprocal(rs[:], sd[:])
                    qn = wkk.tile([P, 512], bf16, tag="qn")
                    nc.vector.tensor_tensor(
                        qn[:].rearrange("p (h d) -> p h d", d=64),
                        ps[:, 0:512].rearrange("p (h d) -> p h d", d=64),
                        rs[:, 0:8, None].to_broadcast([P, 8, 64]), OP.mult)
                    kn = wkk.tile([P, 128], bf16, tag="kn")
                    nc.vector.tensor_tensor(
                        kn[:].rearrange("p (h d) -> p h d", d=64),
                        ps[:, 512:640].rearrange("p (h d) -> p h d", d=64),
                        rs[:, 8:10, None].to_broadcast([P, 2, 64]), OP.mult)
                    # --- v copy (ones cols at 64/129; one strided ACT) ---
                    nc.scalar.activation(
                        vsb[tt][:, 0:130].rearrange(
                            "p (j q) -> p j q", q=65)[:, :, 0:64],
                        ps[:, 640:768].rearrange("p (j q) -> p j q", q=64),
                        AF.Copy)
                    # --- rope: 3 DVE ops per tensor via host-folded tables
                    # tab rows are [A, B, C, -D]; viewed as [P, 2, 2, 32] the
                    # pairs are (A,C) and (B,-D), so
                    # dv = t1*(A,C) - t2*(B,-D) = (t1*A - t2*B | t1*C + t2*D)
                    qr = wkk.tile([P, 512], bf16, tag="qr")
                    kr = wkk.tile([P, 128], bf16, tag="kr")
                    for (src, dst, tab, nh) in ((qn, qr, tq, 8), (kn, kr, tk, 2)):
                        sv = src[:].rearrange("p (h d) -> p h d", d=64)
                        dv = dst[:].rearrange("p (h two f) -> p h two f",
                                              two=2, f=32)
                        tabv = tab[:, tt].rearrange("p (g two) f -> p two g f",
                                                    two=2)
                        t1 = sv[:, :, None, 0:32].to_broadcast([P, nh, 2, 32])
                        t2 = sv[:, :, None, 32:64].to_broadcast([P, nh, 2, 32])
                        AC = tabv[:, 0:1, :, :].to_broadcast([P, nh, 2, 32])
                        BD = tabv[:, 1:2, :, :].to_broadcast([P, nh, 2, 32])
                        u13 = wkk.tile([P, nh, 2, 32], bf16, tag=f"u13_{nh}")
                        u24 = wkk.tile([P, nh, 2, 32], bf16, tag=f"u24_{nh}")
                        nc.vector.tensor_tensor(u13[:], t1, AC, OP.mult)
                        nc.vector.tensor_tensor(u24[:], t2, BD, OP.mult)
                        nc.vector.tensor_tensor(dv, u13[:], u24[:],
                                                OP.subtract)
                    # --- transpose to feature-major (copies on ScalarE) ---
                    for rc in range(4):
                        pt = tp_ps.tile([P, P], bf16, tag="tp")
                        nc.tensor.transpose(pt[:], qr[:, ts(rc, P)], ident[:])
                        nc.scalar.activation(qfm[rc][:, ts(tt, P)], pt[:],
                                             AF.Copy)
                    pt = tp_ps.tile([P, P], bf16, tag="tp")
                    nc.tensor.transpose(pt[:], kr[:], ident[:])
                    nc.scalar.activation(kfm[:, ts(tt, P)], pt[:], AF.Copy)
                    # kswap: partition halves exchanged, built per tile
                    nc.scalar.activation(kswap[64:128, ts(tt, P)], pt[0:64, :],
                                         AF.Copy)
                    nc.scalar.activation(kswap[0:64, ts(tt, P)], pt[64:128, :],
                                         AF.Copy)
            # transition burst: after the last tile's copies drain, these
            # no-consumer transposes run back-to-back (~3.8us sustained PE
            # activity), re-promoting the HAM clock gate to 2.4 GHz before
            # phase 2's fragmented exp-bound stream begins — without this
            # the first ~35us of attention run at 1.2 GHz (promotion needs
            # a fully-busy 3.4us window that fragmented activity never has;
            # LDWEIGHTS pulses do not count as PE-array activity)
            for wu in range(48):
                wt = tp_ps.tile([P, P], bf16, tag="tp")
                nc.tensor.transpose(wt[:], ident[:], ident[:])

        # ---------------- phase 2: attention + out projection ------------
        wopool = ctx.enter_context(tc.tile_pool(name="wo", bufs=1))
        wo_sb = wopool.tile([P, 4, D_MODEL], bf16, name="wo_sb")
        nc.sync.dma_start(wo_sb[:], wo.rearrange("(o p) d -> p o d", p=P))
        with ExitStack() as p2:
            epool = p2.enter_context(tc.tile_pool(name="exp", bufs=4))
            npool = p2.enter_context(tc.tile_pool(name="nrm", bufs=2))
            opool = p2.enter_context(tc.tile_pool(name="osb", bufs=3))
            s_ps = p2.enter_context(
                tc.tile_pool(name="spsum", bufs=2, space="PSUM"))
            y_ps = p2.enter_context(
                tc.tile_pool(name="ypsum", bufs=2, space="PSUM"))

            # out-projection emission: groups for q-chunk qc are spread
            # through qc+1's attention stream so their matmuls fill the
            # tensor-engine gaps of the exp-bound kt pipeline (in-order
            # engine queues: the filler must sit between the stalls).
            pending = []          # (token tile, output half) groups
            emit_ctr = [0]

            def emit_ogroup():
                t, og = pending.pop(0)
                ps_o = s_ps.tile([P, 1024], f32, tag="s")
                for oc2 in range(2):
                    for yc in range(4):
                        nc.tensor.matmul(
                            ps_o[:, ts(oc2, 512)],
                            yfm[yc][:, ts(t, P)],
                            wo_sb[:, yc, 1024 * og + 512 * oc2:
                                  1024 * og + 512 * (oc2 + 1)],
                            start=(yc == 0), stop=(yc == 3))
                ob = opool.tile([P, 1024], f16, tag="ob")
                nc.vector.tensor_copy(ob[:], ps_o[:])
                nc.sync.dma_start(out[ts(t, P), ts(og, 1024)], ob[:])

            # `held` carries the not-yet-emitted PV (+ pair finalizer) of the
            # previous k-tile ACROSS pair boundaries, so scores/exp of the
            # next pair keep both engines fed while the last PV of the
            # previous pair waits on its exp semaphore.
            held = [None]   # (pv_fn, final_fn or None)

            qc_flush = [0, 16]    # [flushes so far in this qc, qc total]

            def flush_held():
                if held[0] is None:
                    return
                pv_fn, final_fn = held[0]
                held[0] = None
                pv_fn()
                if final_fn is not None:
                    final_fn()
                # out-projection groups only flush in the BACK half of the
                # q-chunk: at chunk boundaries the vector queue must stay
                # clear so normalize mults release the y-ring for the next
                # pair's PV (a cast queued ahead of them stalls the whole
                # in-order tensor queue)
                qc_flush[0] += 1
                if (pending and qc_flush[0] > qc_flush[1] // 2
                        and qc_flush[0] % 2 == 0):
                    emit_ogroup()

            for qc in range(QC):
                qc_flush[0] = 0
                qc_flush[1] = 4 * (4 * qc + 4 if causal else 4 * QC)
                for c in range(4):
                    kv = c // 2
                    # A = head 2c (partitions 0:64), B = head 2c+1 (64:128)
                    ksA = kfm if kv == 0 else kswap
                    ksB = kswap if kv == 0 else kfm
                    vsl = slice(65 * kv, 65 * kv + 65)
                    nkt = 4 * qc + 4 if causal else 4 * QC
                    ps_y = y_ps.tile([65, 1024], f32, tag="y")

                    def emit_pv(kt, eg, o, ps_y=ps_y, vsl=vsl, nkt=nkt):
                        nc.tensor.matmul(
                            ps_y[:, o:512], vsb[kt][:, vsl], eg[:, o:512],
                            start=(kt == 0), stop=(kt == nkt - 1))
                        nc.tensor.matmul(
                            ps_y[:, 512 + o:1024], vsb[kt][:, vsl],
                            eg[:, 512 + o:1024],
                            start=(kt == 0), stop=(kt == nkt - 1))

                    def normalize(ps_y=ps_y, c=c, qc=qc):
                        # 1/den via DVE recip + GpSimd partition broadcast
                        # (recip can't read PSUM; vector copy bounces row 64)
                        draw = npool.tile([1, 1024], f32, tag="draw")
                        nc.vector.tensor_copy(draw[0:1, :], ps_y[64:65, :])
                        rec = npool.tile([1, 1024], f32, tag="rec")
                        nc.vector.reciprocal_approx_fast(rec[0:1, :],
                                                         draw[0:1, :])
                        rexp = npool.tile([64, 1024], f32, tag="rexp")
                        nc.gpsimd.partition_broadcast(rexp[:], rec[0:1, :],
                                                      channels=64)
                        nc.vector.tensor_tensor(yfm[c][0:64, ts(qc, 512)],
                                                ps_y[0:64, 0:512],
                                                rexp[:, 0:512], OP.mult)
                        nc.vector.tensor_tensor(yfm[c][64:128, ts(qc, 512)],
                                                ps_y[0:64, 512:1024],
                                                rexp[:, 512:1024], OP.mult)

                    for kt in range(nkt):
                        jl = kt - 4 * qc  # >=0 inside the diagonal quad
                        diag = causal and jl >= 0
                        o = 128 * jl if diag else 0
                        ps_s = s_ps.tile([P, 1024], f32, tag="s")
                        eg = epool.tile([P, 1024], bf16, tag="eg")
                        nc.tensor.matmul(
                            ps_s[:, o:512],
                            ksA[0:64, ts(kt, P)],
                            qfm[c][0:64, 512 * qc + o:512 * (qc + 1)],
                            start=True, stop=True)
                        nc.tensor.matmul(
                            ps_s[:, 512 + o:1024],
                            ksB[64:128, ts(kt, P)],
                            qfm[c][64:128, 512 * qc + o:512 * (qc + 1)],
                            start=True, stop=True)
                        if not diag:
                            nc.scalar.activation(eg[:], ps_s[:], AF.Exp)
                        else:
                            nc.scalar.activation(
                                eg[:].rearrange("p (j q) -> p j q",
                                                q=512)[:, :, o:512],
                                ps_s[:].rearrange("p (j q) -> p j q",
                                                  q=512)[:, :, o:512],
                                AF.Exp)
                            # causal triangle at the diagonal 128-col block
                            nc.gpsimd.affine_select(
                                eg[:].rearrange("p (j q) -> p j q",
                                                q=512)[:, :, o:o + 128],
                                eg[:].rearrange("p (j q) -> p j q",
                                                q=512)[:, :, o:o + 128],
                                pattern=[[0, 2], [1, 128]],
                                compare_op=OP.is_ge,
                                fill=0.0,
                                base=0,
                                channel_multiplier=-1)
                        flush_held()
                        is_last = kt == nkt - 1
                        held[0] = (
                            lambda kt=kt, eg=eg, o=o, f=emit_pv: f(kt, eg, o),
                            normalize if is_last else None)
                # queue this q-chunk's out-projection groups (flushed during
                # qc+1; the final chunk's groups are flushed below)
                for tl in range(4):
                    for og in range(2):
                        pending.append((4 * qc + tl, og))
            flush_held()
            while pending:
                emit_ogroup()

    nc.compile()
    return nc


def _rope_tables(pos, norm_w, scale):
    """Build [P, NT, 4, 32] tables A,B,C,D for out1 = t1*A - t2*B,
    out2 = t1*C + t2*D (NeoX rope with folded norm weight + score scale)."""
    n_tok = pos.shape[0]
    f = np.arange(0, D_HEAD, 2, dtype=np.float64) / D_HEAD
    inv_freq = 1.0 / (ROPE_BASE ** f)                       # [32]
    ang = pos.astype(np.float64)[:, None] * inv_freq[None, :]  # [n, 32]
    cos, sin = np.cos(ang), np.sin(ang)
    w1 = norm_w[:32].astype(np.float64)
    w2 = norm_w[32:].astype(np.float64)
    A = cos * w1 * scale
    Bt = sin * w2 * scale
    C = sin * w1 * scale
    D = cos * w2 * scale
    # D negated: the kernel computes t1*(A,C) - t2*(B,-D) in two fused ops
    tab = np.stack([A, Bt, C, -D], axis=1).astype(np.float32)  # [n, 4, 32]
    return np.ascontiguousarray(
        tab.reshape(n_tok // P, P, 4, 32).transpose(1, 0, 2, 3))


def make_in_maps(x, pos, qkv_w, out_w, q_norm_w, k_norm_w, n_tok=N):
    import ml_dtypes
    bf16 = ml_dtypes.bfloat16

    scale = D_HEAD ** -0.5
    tabq = _rope_tables(pos, q_norm_w, scale).astype(bf16)
    tabk = _rope_tables(pos, k_norm_w, 1.0).astype(bf16)
    wq_all = qkv_w[0:H_Q * D_HEAD].reshape(H_Q, D_HEAD, D_MODEL)
    wk_all = qkv_w[H_Q * D_HEAD:(H_Q + H_KV) * D_HEAD].reshape(
        H_KV, D_HEAD, D_MODEL)
    wv_all = qkv_w[(H_Q + H_KV) * D_HEAD:].reshape(H_KV, D_HEAD, D_MODEL)
    wo_all = out_w.reshape(D_MODEL, H_Q, D_HEAD)

    in_maps = []
    for c in range(NCORES):
        b, hg = divmod(c, 4)
        heads = list(range(8 * hg, 8 * hg + 8))
        kvs = [2 * hg, 2 * hg + 1]
        wsel = np.concatenate([
            wq_all[heads].reshape(512, D_MODEL),
            wk_all[kvs].reshape(128, D_MODEL),
            wv_all[kvs].reshape(128, D_MODEL)], axis=0)    # [768, D]
        in_maps.append({
            "xT": np.ascontiguousarray(x[b].T).astype(bf16),
            "wqkv": np.ascontiguousarray(wsel.T).astype(bf16),
            "wo": np.ascontiguousarray(
                wo_all[:, heads].reshape(D_MODEL, 512).T).astype(bf16),
            "tabq": tabq,
            "tabk": tabk,
        })
    return in_maps


def _reference_host(x, mask, pos, qkv_w, out_w, q_norm_w, k_norm_w):
    """Pure-numpy fallback, used only if the mask is not causal."""
    xx = x.astype(np.float64)
    qkv = xx @ qkv_w.T.astype(np.float64)
    Bsz, Nl, _ = x.shape
    qkv = qkv.reshape(Bsz, Nl, H_Q + 2 * H_KV, D_HEAD).transpose(0, 2, 1, 3)
    q, k, v = (qkv[:, :H_Q], qkv[:, H_Q:H_Q + H_KV], qkv[:, H_Q + H_KV:])

    def rms(t, w):
        var = np.mean(t * t, axis=-1, keepdims=True)
        return t / np.sqrt(var + EPS) * w

    def rope(t):
        f = np.arange(0, D_HEAD, 2) / D_HEAD
        inv = 1.0 / (ROPE_BASE ** f)
        ang = pos.astype(np.float64)[:, None] * inv[None, :]
        cs, sn = np.cos(ang), np.sin(ang)
        t1, t2 = t[..., :32], t[..., 32:]
        return np.concatenate([t1 * cs - t2 * sn, t1 * sn + t2 * cs], axis=-1)

    q, k = rope(rms(q, q_norm_w)), rope(rms(k, k_norm_w))
    qg = q.reshape(Bsz, H_KV, 4, Nl, D_HEAD)
    sc = np.einsum("bhgnd,bhmd->bhgnm", qg, k) * (D_HEAD ** -0.5)
    sc = np.where(mask[None, None, None], -np.inf, sc)
    sc -= sc.max(axis=-1, keepdims=True)
    p = np.exp(sc)
    p /= p.sum(axis=-1, keepdims=True)
    y = np.einsum("bhgnm,bhmd->bhgnd", p, v)
    y = y.reshape(Bsz, H_Q, Nl, D_HEAD).transpose(0, 2, 1, 3).reshape(
        Bsz, Nl, D_MODEL)
    return (y @ out_w.T.astype(np.float64)).astype(np.float32)


_NC_CACHE = {}


def run_on_device(in_maps, n_tok=N, trace=False, trace_kwargs=None):
    import sys
    for p in ("/opt/trn_rl_repo",):
        if p not in sys.path:
            sys.path.insert(0, p)
    from concourse.bass_utils import run_bass_kernel_spmd

    key = n_tok
    if key not in _NC_CACHE:
        _NC_CACHE[key] = build_nc(n_tok)
    nc = _NC_CACHE[key]
    return run_bass_kernel_spmd(
        nc, in_maps, list(range(len(in_maps))), trace=trace,
        **(trace_kwargs or {}))


def kernel(x, mask, pos, qkv_w, out_w, q_norm_w, k_norm_w):
    x = np.asarray(x, dtype=np.float32)
    mask = np.asarray(mask)
    pos = np.asarray(pos)
    causal = bool(
        np.array_equal(mask,
                       np.triu(np.ones((N, N), dtype=bool), k=1)))
    if not causal:
        return _reference_host(x, mask, pos, np.asarray(qkv_w),
                               np.asarray(out_w), np.asarray(q_norm_w),
                               np.asarray(k_norm_w))
    in_maps = make_in_maps(x, pos, np.asarray(qkv_w, dtype=np.float32),
                           np.asarray(out_w, dtype=np.float32),
                           np.asarray(q_norm_w, dtype=np.float32),
                           np.asarray(k_norm_w, dtype=np.float32))
    res = run_on_device(in_maps)
    outs = [r["out"].astype(np.float32) for r in res.results]
    full = np.empty((B, N, D_MODEL), dtype=np.float32)
    for b in range(B):
        full[b] = outs[4 * b] + outs[4 * b + 1] + outs[4 * b + 2] + outs[4 * b + 3]
    return full


# revision 44
# speedup vs baseline: 1.0838x; 1.0464x over previous
"""GQA attention block (B=2, N=2048, D=2048, Hq=32, Hkv=8, d=64) on 8 TRN2 NeuronCores.

Sharding: core c = b*4 + hg  (data-parallel over batch b in {0,1}; tensor-parallel
over 4 head-groups hg, each owning 8 q-heads / 2 kv-heads).  Each core computes a
row-parallel partial of the output projection for its batch; the host sums the 4
partials per batch (fp16 partials).

All matmuls run in bf16 (fp32 matmul costs 4 PE cycles/row vs 1 for bf16);
PSUM accumulation stays fp32 and softmax exp reads fp32 PSUM scores.

Engine-balance notes:
 - ScalarE activation tables: phase 1 uses only {Square, Sqrt, Copy} (one
   sqrt_and_others set), phase 2 only {Exp, Copy} (one exp_and_others set) —
   avoids the ~1.3us per ACT_TABLE_LOAD ping-pong between Ln and Exp sets.
 - The two heads of a pair occupy disjoint 64-partition halves, so their K=64
   score matmuls auto-derive disjoint PE row-group tile_positions and run
   CONCURRENTLY (the pair's scoresT land side by side in one [128,1024] PSUM
   tile, one 1024-wide exp per k-tile).
 - PV runs as one 1024-wide bf16 matmul per k-tile (both heads share the kv
   head, V with an appended ones-column produces y plus the softmax
   denominator); 1/den via DVE reciprocal + GpSimd partition_broadcast (no
   PSUM bank, no broadcast matmul).
 - The out-projection is interleaved per 512-token q-chunk and shares the
   scores' PSUM ring; output DMA'd as fp16 partials.
"""

import numpy as np

D_MODEL = 2048
H_Q, H_KV, D_HEAD = 32, 8, 64
B = 2
N = 2048
ROPE_BASE = 10000.0
EPS = 1e-6
NCORES = 8
P = 128


def _modules():
    import sys

    for p in ("/opt/trn_rl_repo",):
        if p not in sys.path:
            sys.path.insert(0, p)
    import concourse.bass as bass
    import concourse.tile as tile
    from concourse import bacc, mybir
    from concourse.masks import make_identity

    return bass, tile, bacc, mybir, make_identity


def build_nc(n_tok=N, causal=True):
    """Build the single-core SPMD Bass program (identical on all 8 cores)."""
    from contextlib import ExitStack

    bass, tile, bacc, mybir, make_identity = _modules()
    f32 = mybir.dt.float32
    f16 = mybir.dt.float16
    bf16 = mybir.dt.bfloat16
    ts = bass.ts
    AF = mybir.ActivationFunctionType
    OP = mybir.AluOpType

    NT = n_tok // P           # token tiles
    DC = D_MODEL // P         # contraction chunks for qkv proj
    QC = n_tok // 512         # query chunks of 512
    NG = n_tok // 512         # x-load groups (512 tokens each)
    assert QC >= 1 and n_tok % 512 == 0

    nc = bacc.Bacc("TRN2", target_bir_lowering=False, debug=False,
                   num_devices=NCORES)

    xT = nc.dram_tensor("xT", [D_MODEL, n_tok], bf16, kind="ExternalInput").ap()
    wqkv = nc.dram_tensor("wqkv", [D_MODEL, 768], bf16, kind="ExternalInput").ap()
    wo = nc.dram_tensor("wo", [512, D_MODEL], bf16, kind="ExternalInput").ap()
    tabq = nc.dram_tensor("tabq", [P, NT, 4, 32], bf16, kind="ExternalInput").ap()
    tabk = nc.dram_tensor("tabk", [P, NT, 4, 32], bf16, kind="ExternalInput").ap()
    out = nc.dram_tensor("out", [n_tok, D_MODEL], f16, kind="ExternalOutput").ap()

    with ExitStack() as ctx:
        tc = ctx.enter_context(tile.TileContext(nc))

        cpool = ctx.enter_context(tc.tile_pool(name="const", bufs=1))
        # persistent activations (all bf16: they feed matmuls)
        qfm = [cpool.tile([P, n_tok], bf16, name=f"qfm{c}") for c in range(4)]
        kfm = cpool.tile([P, n_tok], bf16, name="kfm")     # [kv0 | kv1] on partitions
        kswap = cpool.tile([P, n_tok], bf16, name="kswap")  # [kv1 | kv0]
        yfm = [cpool.tile([P, n_tok], bf16, name=f"yfm{c}") for c in range(4)]
        vsb = [cpool.tile([P, 130], bf16, name=f"vsb{t}") for t in range(NT)]
        ident = cpool.tile([P, P], bf16, name="ident")
        make_identity(nc, ident[:])
        eps_t = cpool.tile([P, 1], f32, name="eps_t")
        nc.gpsimd.memset(eps_t[:], EPS)
        for t in range(NT):
            nc.gpsimd.memset(vsb[t][:, 64:65], 1.0)
            nc.gpsimd.memset(vsb[t][:, 129:130], 1.0)

        # ---------------- phase 1: qkv + norm + rope + transpose ----------
        with ExitStack() as p1:
            wpool = p1.enter_context(tc.tile_pool(name="wqkv", bufs=1))
            tpool = p1.enter_context(tc.tile_pool(name="tabs", bufs=1))
            xpool = p1.enter_context(tc.tile_pool(name="xg", bufs=3))
            # bufs=3: three token tiles in flight so each tile's serial
            # rmsnorm/rope/transpose chain overlaps the next tiles' matmuls
            wkk = p1.enter_context(tc.tile_pool(name="qkvwork", bufs=3))
            qkv_ps = p1.enter_context(
                tc.tile_pool(name="qkvpsum", bufs=2, space="PSUM"))
            # 4 bufs: the transpose->copy ring cycles ~0.5us per slot; with
            # only 2 the in-order tensor queue blocks at every 3rd transpose
            tp_ps = p1.enter_context(
                tc.tile_pool(name="tppsum", bufs=4, space="PSUM"))

            # x group 0 + first weight chunk issued first so the first qkv
            # matmul can start as early as possible (wq split into 4 tiles)
            xgs = {}
            xgs[0] = xpool.tile([P, DC, 256], bf16, tag="xg", name="xg0")
            nc.sync.dma_start(
                xgs[0][:], xT[:, ts(0, 256)].rearrange("(o p) t -> p o t", p=P))
            wq_sb = [wpool.tile([P, 4, 768], bf16, name=f"wq{i}")
                     for i in range(4)]
            wq_src = wqkv.rearrange("(o p) r -> p o r", p=P)
            for i in range(4):
                nc.sync.dma_start(wq_sb[i][:], wq_src[:, 4 * i:4 * i + 4, :])
            tq = tpool.tile([P, NT, 4, 32], bf16)
            nc.sync.dma_start(tq[:], tabq)
            tk = tpool.tile([P, NT, 4, 32], bf16)
            nc.sync.dma_start(tk[:], tabk)

            # PE warmup: dummy transposes into the tp ring promote the HAM
            # clock gate to 8/8 (2.4 GHz) and keep the PE busy while the
            # first x/w DMAs land (cold PE runs at 1.2 GHz; the activity
            # window is ~3.4us, so idling through the DMAs would re-demote).
            for wu in range(70):
                wt = tp_ps.tile([P, P], bf16, tag="tp")
                nc.tensor.transpose(wt[:], ident[:], ident[:])

            for g in range(2 * NG):
                if g not in xgs:
                    xgs[g] = xpool.tile([P, DC, 256], bf16, tag="xg",
                                        name=f"xg{g}")
                    nc.sync.dma_start(
                        xgs[g][:],
                        xT[:, ts(g, 256)].rearrange("(o p) t -> p o t", p=P))
                xg = xgs[g]
                for lt in range(2):
                    tt = g * 2 + lt
                    ps = qkv_ps.tile([P, 768], f32, tag="qkv")
                    for dc in range(DC):
                        lhsT = xg[:, dc, ts(lt, P)]
                        wsl = wq_sb[dc // 4][:, dc % 4]
                        nc.tensor.matmul(ps[:, 0:512], lhsT, wsl[:, 0:512],
                                         start=(dc == 0), stop=(dc == DC - 1))
                        nc.tensor.matmul(ps[:, 512:768], lhsT, wsl[:, 512:768],
                                         start=(dc == 0), stop=(dc == DC - 1))
                    # --- rmsnorm (Square+Sqrt stay in the sqrt table set) ---
                    sq = wkk.tile([P, 640], f32, tag="sq")
                    nc.scalar.activation(sq[:], ps[:, 0:640], AF.Square)
                    ssq = wkk.tile([P, 10], f32, tag="ssq")
                    nc.vector.reduce_sum(
                        ssq[:], sq[:].rearrange("p (h d) -> p h d", d=64),
                        axis=mybir.AxisListType.X)
                    sd = wkk.tile([P, 10], f32, tag="sd")
                    nc.scalar.activation(sd[:], ssq[:], AF.Sqrt,
                                         bias=eps_t[:], scale=1.0 / 64)
                    rs = wkk.tile([P, 10], f32, tag="rs")
                    nc.vector.reciprocal(rs[:], sd[:])
                    qn = wkk.tile([P, 512], bf16, tag="qn")
                    nc.vector.tensor_tensor(
                        qn[:].rearrange("p (h d) -> p h d", d=64),
                        ps[:, 0:512].rearrange("p (h d) -> p h d", d=64),
                        rs[:, 0:8, None].to_broadcast([P, 8, 64]), OP.mult)
                    kn = wkk.tile([P, 128], bf16, tag="kn")
                    nc.vector.tensor_tensor(
                        kn[:].rearrange("p (h d) -> p h d", d=64),
                        ps[:, 512:640].rearrange("p (h d) -> p h d", d=64),
                        rs[:, 8:10, None].to_broadcast([P, 2, 64]), OP.mult)
                    # --- v copy (ones cols at 64/129; one strided ACT) ---
                    nc.scalar.activation(
                        vsb[tt][:, 0:130].rearrange(
                            "p (j q) -> p j q", q=65)[:, :, 0:64],
                        ps[:, 640:768].rearrange("p (j q) -> p j q", q=64),
                        AF.Copy)
                    # --- rope: 3 DVE ops per tensor via host-folded tables
                    # tab rows are [A, B, C, -D]; viewed as [P, 2, 2, 32] the
                    # pairs are (A,C) and (B,-D), so
                    # dv = t1*(A,C) - t2*(B,-D) = (t1*A - t2*B | t1*C + t2*D)
                    qr = wkk.tile([P, 512], bf16, tag="qr")
                    kr = wkk.tile([P, 128], bf16, tag="kr")
                    for (src, dst, tab, nh) in ((qn, qr, tq, 8), (kn, kr, tk, 2)):
                        sv = src[:].rearrange("p (h d) -> p h d", d=64)
                        dv = dst[:].rearrange("p (h two f) -> p h two f",
                                              two=2, f=32)
                        tabv = tab[:, tt].rearrange("p (g two) f -> p two g f",
                                                    two=2)
                        t1 = sv[:, :, None, 0:32].to_broadcast([P, nh, 2, 32])
                        t2 = sv[:, :, None, 32:64].to_broadcast([P, nh, 2, 32])
                        AC = tabv[:, 0:1, :, :].to_broadcast([P, nh, 2, 32])
                        BD = tabv[:, 1:2, :, :].to_broadcast([P, nh, 2, 32])
                        u13 = wkk.tile([P, nh, 2, 32], bf16, tag=f"u13_{nh}")
                        u24 = wkk.tile([P, nh, 2, 32], bf16, tag=f"u24_{nh}")
                        nc.vector.tensor_tensor(u13[:], t1, AC, OP.mult)
                        nc.vector.tensor_tensor(u24[:], t2, BD, OP.mult)
                        nc.vector.tensor_tensor(dv, u13[:], u24[:],
                                                OP.subtract)
                    # --- transpose to feature-major (copies on ScalarE) ---
                    for rc in range(4):
                        pt = tp_ps.tile([P, P], bf16, tag="tp")
                        nc.tensor.transpose(pt[:], qr[:, ts(rc, P)], ident[:])
                        nc.scalar.activation(qfm[rc][:, ts(tt, P)], pt[:],
                                             AF.Copy)
                    pt = tp_ps.tile([P, P], bf16, tag="tp")
                    nc.tensor.transpose(pt[:], kr[:], ident[:])
                    nc.scalar.activation(kfm[:, ts(tt, P)], pt[:], AF.Copy)
                    # kswap: partition halves exchanged, built per tile
                    nc.scalar.activation(kswap[64:128, ts(tt, P)], pt[0:64, :],
                                         AF.Copy)
                    nc.scalar.activation(kswap[0:64, ts(tt, P)], pt[64:128, :],
                                         AF.Copy)
            # transition burst: after the last tile's copies drain, these
            # no-consumer transposes run back-to-back (~3.8us sustained PE
            # activity), re-promoting the HAM clock gate to 2.4 GHz before
            # phase 2's fragmented exp-bound stream begins — without this
            # the first ~35us of attention run at 1.2 GHz (promotion needs
            # a fully-busy 3.4us window that fragmented activity never has;
            # LDWEIGHTS pulses do not count as PE-array activity)
            for wu in range(48):
                wt = tp_ps.tile([P, P], bf16, tag="tp")
                nc.tensor.transpose(wt[:], ident[:], ident[:])

        # ---------------- phase 2: attention + out projection ------------
        wopool = ctx.enter_context(tc.tile_pool(name="wo", bufs=1))
        wo_sb = wopool.tile([P, 4, D_MODEL], bf16, name="wo_sb")
        nc.sync.dma_start(wo_sb[:], wo.rearrange("(o p) d -> p o d", p=P))
        with ExitStack() as p2:
            epool = p2.enter_context(tc.tile_pool(name="exp", bufs=4))
            npool = p2.enter_context(tc.tile_pool(name="nrm", bufs=2))
            opool = p2.enter_context(tc.tile_pool(name="osb", bufs=3))
            s_ps = p2.enter_context(
                tc.tile_pool(name="spsum", bufs=2, space="PSUM"))
            y_ps = p2.enter_context(
                tc.tile_pool(name="ypsum", bufs=2, space="PSUM"))

            # out-projection emission: groups for q-chunk qc are spread
            # through qc+1's attention stream so their matmuls fill the
            # tensor-engine gaps of the exp-bound kt pipeline (in-order
            # engine queues: the filler must sit between the stalls).
            pending = []          # (token tile, output half) groups
            emit_ctr = [0]

            def emit_ogroup():
                t, og = pending.pop(0)
                ps_o = s_ps.tile([P, 1024], f32, tag="s")
                for oc2 in range(2):
                    for yc in range(4):
                        nc.tensor.matmul(
                            ps_o[:, ts(oc2, 512)],
                            yfm[yc][:, ts(t, P)],
                            wo_sb[:, yc, 1024 * og + 512 * oc2:
                                  1024 * og + 512 * (oc2 + 1)],
                            start=(yc == 0), stop=(yc == 3))
                ob = opool.tile([P, 1024], f16, tag="ob")
                nc.vector.tensor_copy(ob[:], ps_o[:])
                nc.sync.dma_start(out[ts(t, P), ts(og, 1024)], ob[:])

            # `held` carries the not-yet-emitted PV (+ pair finalizer) of the
            # previous k-tile ACROSS pair boundaries, so scores/exp of the
            # next pair keep both engines fed while the last PV of the
            # previous pair waits on its exp semaphore.
            held = []   # FIFO of (pv_fn, final_fn or None); depth-2 skew

            qc_flush = [0, 16]    # [flushes so far in this qc, qc total]

            def flush_held():
                if not held:
                    return
                pv_fn, final_fn = held.pop(0)
                pv_fn()
                if final_fn is not None:
                    final_fn()
                # out-projection groups only flush in the BACK half of the
                # q-chunk: at chunk boundaries the vector queue must stay
                # clear so normalize mults release the y-ring for the next
                # pair's PV (a cast queued ahead of them stalls the whole
                # in-order tensor queue)
                qc_flush[0] += 1
                if (pending and qc_flush[0] > qc_flush[1] // 2
                        and qc_flush[0] % 2 == 0):
                    emit_ogroup()

            for qc in range(QC):
                qc_flush[0] = 0
                qc_flush[1] = 4 * (4 * qc + 4 if causal else 4 * QC)
                for c in range(4):
                    kv = c // 2
                    # A = head 2c (partitions 0:64), B = head 2c+1 (64:128)
                    ksA = kfm if kv == 0 else kswap
                    ksB = kswap if kv == 0 else kfm
                    vsl = slice(65 * kv, 65 * kv + 65)
                    nkt = 4 * qc + 4 if causal else 4 * QC
                    ps_y = y_ps.tile([65, 1024], f32, tag="y")

                    def emit_pv(kt, eg, o, ps_y=ps_y, vsl=vsl, nkt=nkt):
                        nc.tensor.matmul(
                            ps_y[:, o:512], vsb[kt][:, vsl], eg[:, o:512],
                            start=(kt == 0), stop=(kt == nkt - 1))
                        nc.tensor.matmul(
                            ps_y[:, 512 + o:1024], vsb[kt][:, vsl],
                            eg[:, 512 + o:1024],
                            start=(kt == 0), stop=(kt == nkt - 1))

                    def normalize(ps_y=ps_y, c=c, qc=qc):
                        # 1/den via DVE recip + GpSimd partition broadcast
                        # (recip can't read PSUM; vector copy bounces row 64)
                        draw = npool.tile([1, 1024], f32, tag="draw")
                        nc.vector.tensor_copy(draw[0:1, :], ps_y[64:65, :])
                        rec = npool.tile([1, 1024], f32, tag="rec")
                        nc.vector.reciprocal_approx_fast(rec[0:1, :],
                                                         draw[0:1, :])
                        rexp = npool.tile([64, 1024], f32, tag="rexp")
                        nc.gpsimd.partition_broadcast(rexp[:], rec[0:1, :],
                                                      channels=64)
                        nc.vector.tensor_tensor(yfm[c][0:64, ts(qc, 512)],
                                                ps_y[0:64, 0:512],
                                                rexp[:, 0:512], OP.mult)
                        nc.vector.tensor_tensor(yfm[c][64:128, ts(qc, 512)],
                                                ps_y[0:64, 512:1024],
                                                rexp[:, 512:1024], OP.mult)

                    for kt in range(nkt):
                        jl = kt - 4 * qc  # >=0 inside the diagonal quad
                        diag = causal and jl >= 0
                        o = 128 * jl if diag else 0
                        ps_s = s_ps.tile([P, 1024], f32, tag="s")
                        eg = epool.tile([P, 1024], bf16, tag="eg")
                        nc.tensor.matmul(
                            ps_s[:, o:512],
                            ksA[0:64, ts(kt, P)],
                            qfm[c][0:64, 512 * qc + o:512 * (qc + 1)],
                            start=True, stop=True)
                        nc.tensor.matmul(
                            ps_s[:, 512 + o:1024],
                            ksB[64:128, ts(kt, P)],
                            qfm[c][64:128, 512 * qc + o:512 * (qc + 1)],
                            start=True, stop=True)
                        if not diag:
                            nc.scalar.activation(eg[:], ps_s[:], AF.Exp)
                        else:
                            nc.scalar.activation(
                                eg[:].rearrange("p (j q) -> p j q",
                                                q=512)[:, :, o:512],
                                ps_s[:].rearrange("p (j q) -> p j q",
                                                  q=512)[:, :, o:512],
                                AF.Exp)
                            # causal triangle at the diagonal 128-col block
                            nc.gpsimd.affine_select(
                                eg[:].rearrange("p (j q) -> p j q",
                                                q=512)[:, :, o:o + 128],
                                eg[:].rearrange("p (j q) -> p j q",
                                                q=512)[:, :, o:o + 128],
                                pattern=[[0, 2], [1, 128]],
                                compare_op=OP.is_ge,
                                fill=0.0,
                                base=0,
                                channel_multiplier=-1)
                        if len(held) >= 2:
                            flush_held()
                        is_last = kt == nkt - 1
                        held.append((
                            lambda kt=kt, eg=eg, o=o, f=emit_pv: f(kt, eg, o),
                            normalize if is_last else None))
                # queue this q-chunk's out-projection groups (flushed during
                # qc+1; the final chunk's groups are flushed below)
                for tl in range(4):
                    for og in range(2):
                        pending.append((4 * qc + tl, og))
            while held:
                flush_held()
            while pending:
                emit_ogroup()

    nc.compile()
    return nc


def _rope_tables(pos, norm_w, scale):
    """Build [P, NT, 4, 32] tables A,B,C,D for out1 = t1*A - t2*B,
    out2 = t1*C + t2*D (NeoX rope with folded norm weight + score scale)."""
    n_tok = pos.shape[0]
    f = np.arange(0, D_HEAD, 2, dtype=np.float64) / D_HEAD
    inv_freq = 1.0 / (ROPE_BASE ** f)                       # [32]
    ang = pos.astype(np.float64)[:, None] * inv_freq[None, :]  # [n, 32]
    cos, sin = np.cos(ang), np.sin(ang)
    w1 = norm_w[:32].astype(np.float64)
    w2 = norm_w[32:].astype(np.float64)
    A = cos * w1 * scale
    Bt = sin * w2 * scale
    C = sin * w1 * scale
    D = cos * w2 * scale
    # D negated: the kernel computes t1*(A,C) - t2*(B,-D) in two fused ops
    tab = np.stack([A, Bt, C, -D], axis=1).astype(np.float32)  # [n, 4, 32]
    return np.ascontiguousarray(
        tab.reshape(n_tok // P, P, 4, 32).transpose(1, 0, 2, 3))


def make_in_maps(x, pos, qkv_w, out_w, q_norm_w, k_norm_w, n_tok=N):
    import ml_dtypes
    bf16 = ml_dtypes.bfloat16

    scale = D_HEAD ** -0.5
    tabq = _rope_tables(pos, q_norm_w, scale).astype(bf16)
    tabk = _rope_tables(pos, k_norm_w, 1.0).astype(bf16)
    wq_all = qkv_w[0:H_Q * D_HEAD].reshape(H_Q, D_HEAD, D_MODEL)
    wk_all = qkv_w[H_Q * D_HEAD:(H_Q + H_KV) * D_HEAD].reshape(
        H_KV, D_HEAD, D_MODEL)
    wv_all = qkv_w[(H_Q + H_KV) * D_HEAD:].reshape(H_KV, D_HEAD, D_MODEL)
    wo_all = out_w.reshape(D_MODEL, H_Q, D_HEAD)

    in_maps = []
    for c in range(NCORES):
        b, hg = divmod(c, 4)
        heads = list(range(8 * hg, 8 * hg + 8))
        kvs = [2 * hg, 2 * hg + 1]
        wsel = np.concatenate([
            wq_all[heads].reshape(512, D_MODEL),
            wk_all[kvs].reshape(128, D_MODEL),
            wv_all[kvs].reshape(128, D_MODEL)], axis=0)    # [768, D]
        in_maps.append({
            "xT": np.ascontiguousarray(x[b].T).astype(bf16),
            "wqkv": np.ascontiguousarray(wsel.T).astype(bf16),
            "wo": np.ascontiguousarray(
                wo_all[:, heads].reshape(D_MODEL, 512).T).astype(bf16),
            "tabq": tabq,
            "tabk": tabk,
        })
    return in_maps


def _reference_host(x, mask, pos, qkv_w, out_w, q_norm_w, k_norm_w):
    """Pure-numpy fallback, used only if the mask is not causal."""
    xx = x.astype(np.float64)
    qkv = xx @ qkv_w.T.astype(np.float64)
    Bsz, Nl, _ = x.shape
    qkv = qkv.reshape(Bsz, Nl, H_Q + 2 * H_KV, D_HEAD).transpose(0, 2, 1, 3)
    q, k, v = (qkv[:, :H_Q], qkv[:, H_Q:H_Q + H_KV], qkv[:, H_Q + H_KV:])

    def rms(t, w):
        var = np.mean(t * t, axis=-1, keepdims=True)
        return t / np.sqrt(var + EPS) * w

    def rope(t):
        f = np.arange(0, D_HEAD, 2) / D_HEAD
        inv = 1.0 / (ROPE_BASE ** f)
        ang = pos.astype(np.float64)[:, None] * inv[None, :]
        cs, sn = np.cos(ang), np.sin(ang)
        t1, t2 = t[..., :32], t[..., 32:]
        return np.concatenate([t1 * cs - t2 * sn, t1 * sn + t2 * cs], axis=-1)

    q, k = rope(rms(q, q_norm_w)), rope(rms(k, k_norm_w))
    qg = q.reshape(Bsz, H_KV, 4, Nl, D_HEAD)
    sc = np.einsum("bhgnd,bhmd->bhgnm", qg, k) * (D_HEAD ** -0.5)
    sc = np.where(mask[None, None, None], -np.inf, sc)
    sc -= sc.max(axis=-1, keepdims=True)
    p = np.exp(sc)
    p /= p.sum(axis=-1, keepdims=True)
    y = np.einsum("bhgnm,bhmd->bhgnd", p, v)
    y = y.reshape(Bsz, H_Q, Nl, D_HEAD).transpose(0, 2, 1, 3).reshape(
        Bsz, Nl, D_MODEL)
    return (y @ out_w.T.astype(np.float64)).astype(np.float32)


_NC_CACHE = {}


def run_on_device(in_maps, n_tok=N, trace=False, trace_kwargs=None):
    import sys
    for p in ("/opt/trn_rl_repo",):
        if p not in sys.path:
            sys.path.insert(0, p)
    from concourse.bass_utils import run_bass_kernel_spmd

    key = n_tok
    if key not in _NC_CACHE:
        _NC_CACHE[key] = build_nc(n_tok)
    nc = _NC_CACHE[key]
    return run_bass_kernel_spmd(
        nc, in_maps, list(range(len(in_maps))), trace=trace,
        **(trace_kwargs or {}))


def kernel(x, mask, pos, qkv_w, out_w, q_norm_w, k_norm_w):
    x = np.asarray(x, dtype=np.float32)
    mask = np.asarray(mask)
    pos = np.asarray(pos)
    causal = bool(
        np.array_equal(mask,
                       np.triu(np.ones((N, N), dtype=bool), k=1)))
    if not causal:
        return _reference_host(x, mask, pos, np.asarray(qkv_w),
                               np.asarray(out_w), np.asarray(q_norm_w),
                               np.asarray(k_norm_w))
    in_maps = make_in_maps(x, pos, np.asarray(qkv_w, dtype=np.float32),
                           np.asarray(out_w, dtype=np.float32),
                           np.asarray(q_norm_w, dtype=np.float32),
                           np.asarray(k_norm_w, dtype=np.float32))
    res = run_on_device(in_maps)
    outs = [r["out"].astype(np.float32) for r in res.results]
    full = np.empty((B, N, D_MODEL), dtype=np.float32)
    for b in range(B):
        full[b] = outs[4 * b] + outs[4 * b + 1] + outs[4 * b + 2] + outs[4 * b + 3]
    return full
